# revision 22
# baseline (speedup 1.0000x reference)
"""Curvphormer GNN layer as a Bass/Tile SPMD kernel for TRN2.

Design (per core c of NCORES, equal node ranges of W windows x 128 nodes;
edges sharded by src-window range):
 - Phase A: fused-LN q/k/v build for own node range (LN folded into matmuls);
   q/k/v tables in bf16. AllGather(k), AllGather(v).
 - Pass 1 (edges grouped by 256-node tgt windows, 16-tile blocks):
   q[src] per-tile indirect-DMA gather (own table); k[tgt] via one-hot
   MATMULS (host-streamed transposed one-hots x streamed k-window slabs) --
   no per-edge DMA; scores = q.k/4 + curv@Wc + bc (block-fused DVE ops),
   ex = exp(score) (max-free softmax: scores are O(1) by construction);
   segment-sum of ex by tgt via one-hot matmuls into per-wide-window PSUM ->
   SBUF denominator table. Padding handled by an out-of-range one-hot index.
 - AllReduce(denominators) -> full [N,H] table on every core.
 - Pass 1.5 (same tgt-grouped order): v[tgt] and 1/den[tgt] gathered by the
   same one-hot matmuls; msg[slot] = ex * v/den written contiguously to DRAM
   in pass-1 slot order. No per-edge DMA.
 - Pass 2 (edges grouped by own src-window, fixed T2W tiles per window):
   single per-tile indirect-DMA gather of msg rows (by pass-1 position);
   mask + aggregate transposed agg via one-hot bf16 matmuls in PSUM/window.
 - Phase D (fused per window): out = x1 + FFN(LN2(x1)), x1 = x + agg@Wo + bo;
   FFN/attn-out matmuls in bf16.

Indirect DMA note: the hardware honors only single-column [128,1] offset APs
(one offset per partition per call); multi-column offset batching silently
degrades to consecutive-row reads. All remaining indirect gathers therefore
use per-tile single-column offsets; everything else was restructured into
one-hot matmuls / contiguous streams.

Timing: chained-dispatch marginal. A single dispatch over the axon tunnel has
~75 ms of fixed client<->device round-trip latency that is unrelated to kernel
execution; we measure T(1) and T(1+B) where the B extra executions are chained
back-to-back on device (each feeding its donated output buffer to the next
call), and report (T(1+B)-T(1))/B -- the steady-state hardware execution time
per run.
"""

import sys
if "/opt/trn_rl_repo" not in sys.path:
    sys.path.insert(0, "/opt/trn_rl_repo")

import numpy as np

import concourse.bass as bass
import concourse.mybir as mybir
from concourse.masks import make_identity

F32 = mybir.dt.float32
BF16 = mybir.dt.bfloat16
I32 = mybir.dt.int32

D = 128
H = 8
HD = 16
LN_EPS = 1e-5
NOMATCH = 300.0  # one-hot index for padded slots: never matches iota < 256


class P:
    """Static program parameters (identical across cores -> SPMD safe)."""

    def __init__(self, ncores, W, T1, T2W):
        self.ncores = ncores
        self.W = W              # windows (of 128 nodes) per core
        self.T1 = T1            # pass-1 tiles (128 edges each) per core
        self.T2W = T2W          # pass-2 tiles per window
        self.nodes_pc = W * 128
        self.npad = ncores * W * 128
        self.T2 = W * T2W


def _bf16(a):
    import ml_dtypes
    return np.asarray(a, dtype=ml_dtypes.bfloat16)


# --------------------------------------------------------------------------
# Host-side preprocessing
# --------------------------------------------------------------------------

def host_prep(x, edge_index, curv, weights, ncores, W):
    """Build per-core input maps. weights: dict with raw reference weights."""
    N = x.shape[0]
    E = edge_index.shape[1]
    nodes_pc = W * 128
    npad = ncores * nodes_pc
    assert npad >= N

    src = np.asarray(edge_index[0], dtype=np.int64)
    tgt = np.asarray(edge_index[1], dtype=np.int64)
    x_pad = np.zeros((npad, D), dtype=np.float32)
    x_pad[:N] = x

    core_of = (src // 128) // W
    order_by_core = np.argsort(core_of, kind="stable")
    counts = np.bincount(core_of, minlength=ncores)
    splits = np.split(order_by_core, np.cumsum(counts)[:-1])

    # pass-1: edges grouped by 256-node wide tgt-windows, padded to a fixed
    # tile count per wide-window (static, SPMD-uniform).
    NWW = (ncores * W + 1) // 2  # wide windows of 256 nodes
    T1W = 0
    for c in range(ncores):
        cnt = np.bincount(tgt[splits[c]] // 256, minlength=NWW)
        T1W = max(T1W, int(np.ceil(cnt.max() / 128)))
    T1 = NWW * T1W
    # pass-2: max tiles per (core, window)
    T2W = 0
    for c in range(ncores):
        e_c = splits[c]
        w_loc = (src[e_c] // 128) - c * W
        cnt = np.bincount(w_loc, minlength=W)
        T2W = max(T2W, int(np.ceil(cnt.max() / 128)))
    # paired pass-2: per-window entries (pairs + singles) observed at
    # ~0.54x sub-tile count; 0.55x + ceil gives ~16% headroom.
    T2W = 2 * int(np.ceil(T2W * 0.55))
    T2 = W * T2W

    pp = P(ncores, W, T1, T2W)
    pp.NWW = NWW
    pp.T1W = T1W

    # LN-folded weights (host)
    g1, be1, g2, be2 = weights["g1"], weights["be1"], weights["g2"], weights["be2"]

    def fold(Wm, b):
        Wp = (g1[:, None] * Wm).astype(np.float32)
        r1 = Wp.sum(axis=0).astype(np.float32)
        b2 = (be1 @ Wm + b).astype(np.float32)
        return Wp, r1, b2

    wq, r1q, bq2 = fold(weights["Wq"], weights["bq"])
    wk, r1k, bk2 = fold(weights["Wk"], weights["bk"])
    wv, r1v, bv2 = fold(weights["Wv"], weights["bv"])
    w1 = (g2[:, None] * weights["W1"]).astype(np.float32)
    r11 = w1.sum(axis=0).astype(np.float32)
    b12 = (be2 @ weights["W1"] + weights["b1"]).astype(np.float32)

    common = {
        "wq": wq, "wk": wk, "wv": wv,
        "wc": _bf16(weights["Wc"]),
        "wo": _bf16(weights["Wo"]),
        "w1": _bf16(w1),
        "w2": _bf16(np.ascontiguousarray(
            weights["W2"].astype(np.float32).reshape(4, 128, D)
            .transpose(1, 0, 2).reshape(128, 4 * D))),
        "r1q": r1q[None, :], "r1k": r1k[None, :], "r1v": r1v[None, :],
        "bq2": bq2[None, :], "bk2": bk2[None, :], "bv2": bv2[None, :],
        "bc_b": np.tile(weights["bc"].astype(np.float32)[None, :], (128, 1)),
        "bo_r": _bf16(weights["bo"])[None, :],
        "r11": _bf16(r11)[None, :], "b12": _bf16(b12)[None, :],
        "b2_r": _bf16(weights["b2"])[None, :],
        "ones_r": _bf16(np.ones((1, D), np.float32)),
        "iota256": np.tile(np.arange(256, dtype=np.float32)[None, :], (128, 1)),
        "iota128": np.tile(np.arange(128, dtype=np.float32)[None, :], (128, 1)),
    }

    in_maps = []
    for c in range(ncores):
        e_c = splits[c]
        L = len(e_c)
        # ---- pass 1: group by wide tgt-window, fixed T1W tiles each ----
        NWW, T1W = pp.NWW, pp.T1W
        S1 = T1 * 128
        tgt1 = np.zeros(S1, np.int64)
        src1 = np.zeros(S1, np.int64)
        real1 = np.zeros(S1, bool)
        slot1_of_edge = np.zeros(E, np.int64)
        ww_of = tgt[e_c] // 256
        w_of_all = (src // 128) - c * W
        # per-window entry lists for paired pass-2 gathers:
        # entry = (rowpair, liveA(edge or -1), liveB(edge or -1))
        entries_w = [[] for _ in range(W)]
        for ww in range(NWW):
            ew = e_c[ww_of == ww]
            base_t = ww * T1W
            # domino (pair) cells start at even GLOBAL tile parity
            c0 = 0 if (base_t % 2 == 0) else 1
            dom_cols = [cc for cc in (c0, c0 + 2) if cc + 1 < T1W]
            dominoes = [(p, cc) for p in range(128) for cc in dom_cols]
            used = np.zeros((T1W, 128), bool)
            ndom = 0
            placed = []  # (edge, lt, p)
            wv = w_of_all[ew]
            order = np.argsort(wv, kind="stable")
            ew_s = ew[order]; wv_s = wv[order]
            i = 0
            singles = []
            while i < len(ew_s):
                jx = i
                while jx < len(ew_s) and wv_s[jx] == wv_s[i]:
                    jx += 1
                grp = ew_s[i:jx]
                wloc = int(wv_s[i])
                gi = 0
                while gi + 1 < len(grp) and ndom < len(dominoes):
                    p, cc = dominoes[ndom]; ndom += 1
                    eA, eB = int(grp[gi]), int(grp[gi + 1]); gi += 2
                    placed.append((eA, cc, p)); placed.append((eB, cc + 1, p))
                    used[cc, p] = True; used[cc + 1, p] = True
                    rp = (p * T1 + base_t + cc) // 2
                    entries_w[wloc].append((rp, eA, eB))
                for e1 in grp[gi:]:
                    singles.append((int(e1), wloc))
                i = jx
            free_cells = [(lt, p) for lt in range(T1W) for p in range(128)
                          if not used[lt, p]]
            assert len(free_cells) >= len(singles)
            for (e1, wloc), (lt, p) in zip(singles, free_cells):
                placed.append((e1, lt, p))
                tg = base_t + lt
                rp = (p * T1 + tg) // 2
                if tg % 2 == 0:
                    entries_w[wloc].append((rp, e1, -1))
                else:
                    entries_w[wloc].append((rp, -1, e1))
            for e1, lt, p in placed:
                s_ = (base_t + lt) * 128 + p
                tgt1[s_] = tgt[e1]
                src1[s_] = src[e1]
                real1[s_] = True
                slot1_of_edge[e1] = s_

        wwin1 = np.repeat(np.arange(T1) // T1W, 128)  # wide window per slot
        tgt_rel = np.where(real1, tgt1 - wwin1 * 256, NOMATCH)
        assert tgt_rel.min() >= 0 and tgt_rel.max() <= NOMATCH

        curv1 = np.zeros((S1, D), np.float32)
        if L:
            curv1[real1] = curv[np.concatenate(
                [e_c[ww_of == ww] for ww in range(NWW)])]
        # [128 d, T1*128] partition-major layout: row d, col t*128+e
        curv1t = _bf16(np.ascontiguousarray(
            curv1.reshape(T1, 128, D).transpose(2, 0, 1).reshape(D, T1 * 128)))

        def lay(a, T):  # [T*128] -> [128, T]
            return np.ascontiguousarray(a.reshape(T, 128).T)

        qi = lay(np.where(real1, src1 - c * nodes_pc, 0).astype(np.int32), T1)
        trel = lay(tgt_rel.astype(np.float32), T1)
        # transposed one-hots for the k-gather matmuls: ohT[n, slot] =
        # (tgt_rel[slot] == n) / (== n+128); slot s maps to column s
        # (device tile t = s//128, partition e = s%128 -> col t*128+e = s).
        import ml_dtypes as _mld
        ohT_lo = np.zeros((128, S1), _mld.bfloat16)
        ohT_hi = np.zeros((128, S1), _mld.bfloat16)
        s_idx = np.arange(S1)
        reli = tgt_rel.astype(np.int64)
        m_lo = real1 & (reli < 128)
        m_hi = real1 & (reli >= 128) & (reli < 256)
        ohT_lo[reli[m_lo], s_idx[m_lo]] = 1
        ohT_hi[reli[m_hi] - 128, s_idx[m_hi]] = 1

        # ---- pass 2: paired entries per src-window ----
        # pair-tile m of window w covers sub-tiles (2m, 2m+1); one 512B-row
        # descriptor per partition fetches both msgs; dead sub-slots get
        # srcl2=NOMATCH so their junk msg scatters into a zero one-hot.
        T2P = T2 // 2
        T2WP = T2W // 2
        S2 = T2 * 128
        exp_pos = np.zeros(T2P * 128, np.int64)
        sl2 = np.full(S2, NOMATCH, np.float32)
        for w in range(W):
            ents = entries_w[w]
            assert len(ents) <= T2WP * 128, (len(ents), T2WP * 128)
            for i, (rp, eA, eB) in enumerate(ents):
                m_ = w * T2WP + i // 128
                p2 = i % 128
                exp_pos[m_ * 128 + p2] = rp
                for half, e1 in ((0, eA), (1, eB)):
                    if e1 >= 0:
                        sl2[(2 * m_ + half) * 128 + p2] = \
                            src[e1] - (c * W + w) * 128
        expos = lay(exp_pos.astype(np.int32), T2P)
        srcl2 = lay(sl2, T2)

        x_own = np.ascontiguousarray(x_pad[c * nodes_pc:(c + 1) * nodes_pc])
        xT_own = np.ascontiguousarray(x_own.T)

        m = dict(common)
        m.update({
            "x_own": x_own, "xT_own": xT_own,
            "curv1t": curv1t,
            "qi": qi, "trel": trel,
            "ohT_lo": ohT_lo, "ohT_hi": ohT_hi,
            "expos": expos, "srcl2": srcl2,
        })
        in_maps.append(m)

    return pp, in_maps


# --------------------------------------------------------------------------
# Device program
# --------------------------------------------------------------------------

def declare_io(nc, pp):
    """Declare all ExternalInput/Output dram tensors; returns dict of APs."""
    t = {}

    def din(name, shape, dt=F32):
        t[name] = nc.dram_tensor(name, list(shape), dt, kind="ExternalInput").ap()

    W, T1, T2 = pp.W, pp.T1, pp.T2
    din("x_own", (pp.nodes_pc, D)); din("xT_own", (D, pp.nodes_pc))
    din("curv1t", (D, T1 * 128), BF16)
    din("qi", (128, T1), I32)
    din("trel", (128, T1))
    din("ohT_lo", (128, T1 * 128), BF16); din("ohT_hi", (128, T1 * 128), BF16)
    din("expos", (128, T2 // 2), I32)
    din("srcl2", (128, T2))
    for n, shp, dt in [("wq", (D, D), F32), ("wk", (D, D), F32),
                       ("wv", (D, D), F32),
                       ("wc", (D, H), BF16), ("wo", (D, D), BF16),
                       ("w1", (D, 4 * D), BF16), ("w2", (D, 4 * D), BF16),
                       ("r1q", (1, D), F32), ("r1k", (1, D), F32),
                       ("r1v", (1, D), F32),
                       ("bq2", (1, D), F32), ("bk2", (1, D), F32),
                       ("bv2", (1, D), F32),
                       ("bc_b", (128, H), F32), ("bo_r", (1, D), BF16),
                       ("r11", (1, 4 * D), BF16), ("b12", (1, 4 * D), BF16),
                       ("b2_r", (1, D), BF16),
                       ("ones_r", (1, D), BF16),
                       ("iota256", (128, 256), F32),
                       ("iota128", (128, 128), F32)]:
        din(n, shp, dt)
    t["out"] = nc.dram_tensor("out", [pp.nodes_pc, D], F32,
                              kind="ExternalOutput").ap()
    return t


def build(tc, t, pp):
    import os as _os
    _abl_no_coll = bool(_os.environ.get("ABL_NO_COLL"))
    _abl_no_p15 = bool(_os.environ.get("ABL_NO_P15"))
    nc = tc.nc
    _rr = [0]

    def ind_dma(out, in_, off_ap):
        import os
        if os.environ.get("ABL_NO_GATHER"):
            return None
        inst = nc.gpsimd.indirect_dma_start(
            out=out, out_offset=None, in_=in_,
            in_offset=bass.IndirectOffsetOnAxis(ap=off_ap, axis=0))
        q = _rr[0] % 4
        _rr[0] += 1
        if q:
            inst.ins.queue = f"qPoolDynamic{q}"
        return inst
    W, T1, T2W, T2 = pp.W, pp.T1, pp.T2W, pp.T2
    NW = pp.ncores * W  # total windows (392)
    rg = [list(range(pp.ncores))]
    from contextlib import ExitStack
    ctx = ExitStack()

    # internal DRAM
    q_own_d, _ = tc.tile([pp.nodes_pc, D], BF16, space="DRAM", name="q_own_d")
    kv_own_d, _ = tc.tile([pp.nodes_pc, 2 * D], BF16, space="DRAM",
                          name="kv_own_d")
    kv_full, _ = tc.tile([pp.npad, 2 * D], BF16, space="DRAM",
                         addr_space="Shared", name="kv_full")
    den_d, _ = tc.tile([NW * 128, H], F32, space="DRAM", name="den_d")
    den_full_d, _ = tc.tile([NW * 128, H], F32, space="DRAM",
                            addr_space="Shared", name="den_full_d")
    msg_d, _ = tc.tile([128 * (T1 // 2), 256], BF16, space="DRAM",
                       name="msg_d")

    const = ctx.enter_context(tc.tile_pool(name="const", bufs=1))

    def load_const(name, dt=None, src=None):
        ap = t[name] if src is None else src
        shp = list(ap.shape)
        tl = const.tile(shp, dt or ap.dtype, name=f"c_{name}")
        nc.sync.dma_start(tl[:], ap[:])
        return tl

    wq_s = load_const("wq"); wk_s = load_const("wk"); wv_s = load_const("wv")
    wc_s = load_const("wc"); wo_s = load_const("wo"); w1_s = load_const("w1")
    w2_s = load_const("w2")
    r1q_s = load_const("r1q"); r1k_s = load_const("r1k"); r1v_s = load_const("r1v")
    bq2_s = load_const("bq2"); bk2_s = load_const("bk2"); bv2_s = load_const("bv2")
    bc_s = load_const("bc_b"); bo_s = load_const("bo_r")
    r11_s = load_const("r11"); b12_s = load_const("b12"); b2_s = load_const("b2_r")
    ones_s = load_const("ones_r")
    qi_s = load_const("qi")
    expos_s = load_const("expos")
    srcl2_f = load_const("srcl2")
    trel_f = load_const("trel")
    iota256_f = load_const("iota256")
    iota128_f = load_const("iota128")

    ident = const.tile([128, 128], F32, name="ident")
    make_identity(nc, ident[:])
    ident_b = const.tile([128, 128], BF16, name="ident_b")
    nc.vector.tensor_copy(out=ident_b[:], in_=ident[:])
    eps_col = const.tile([128, 1], F32, name="eps_col")
    nc.vector.memset(eps_col[:], LN_EPS)

    # bf16 copies for the one-hot / message paths
    trel_s = const.tile([128, T1], BF16, name="trel_b")
    nc.vector.tensor_copy(out=trel_s[:], in_=trel_f[:])
    iota256_s = const.tile([128, 256], BF16, name="iota256_b")
    nc.vector.tensor_copy(out=iota256_s[:], in_=iota256_f[:])
    iota128_s = const.tile([128, 128], BF16, name="iota128_b")
    nc.vector.tensor_copy(out=iota128_s[:], in_=iota128_f[:])
    srcl2_s = const.tile([128, T2], BF16, name="srcl2_b")
    nc.vector.tensor_copy(out=srcl2_s[:], in_=srcl2_f[:])

    # residents
    v_res = const.tile([128, W * 128], BF16, name="v_res")
    ex_sb = const.tile([128, T1 * H], BF16, name="ex_sb")
    den_tab = const.tile([128, (NW + 1) * H], F32, name="den_tab")
    nc.vector.memset(den_tab[:], 0.0)

    # ---------------- Phase A: q/k/v for own windows ----------------
    with tc.tile_pool(name="pA", bufs=2) as pA, \
         tc.tile_pool(name="pAp", bufs=1, space="PSUM") as pAp:
        for w in range(W):
            xw = pA.tile([128, 128], F32, tag="xw")
            nc.sync.dma_start(xw[:], t["x_own"][w * 128:(w + 1) * 128, :])
            xTw = pA.tile([128, 128], F32, tag="xTw")
            nc.sync.dma_start(xTw[:], t["xT_own"][:, w * 128:(w + 1) * 128])
            # stats
            s1 = pA.tile([128, 1], F32, tag="s1")
            nc.vector.tensor_reduce(out=s1[:], in_=xw[:],
                                    axis=mybir.AxisListType.X,
                                    op=mybir.AluOpType.add)
            sq = pA.tile([128, 128], F32, tag="sq")
            nc.scalar.activation(out=sq[:], in_=xw[:],
                                 func=mybir.ActivationFunctionType.Square)
            s2 = pA.tile([128, 1], F32, tag="s2")
            nc.vector.tensor_reduce(out=s2[:], in_=sq[:],
                                    axis=mybir.AxisListType.X,
                                    op=mybir.AluOpType.add)
            mcol = pA.tile([128, 1], F32, tag="mcol")
            nc.vector.tensor_scalar_mul(mcol[:], s1[:], 1.0 / 128.0)
            m2c = pA.tile([128, 1], F32, tag="m2c")
            nc.vector.tensor_tensor(out=m2c[:], in0=mcol[:], in1=mcol[:],
                                    op=mybir.AluOpType.mult)
            var = pA.tile([128, 1], F32, tag="var")
            nc.vector.scalar_tensor_tensor(out=var[:], in0=s2[:],
                                           scalar=1.0 / 128.0, in1=m2c[:],
                                           op0=mybir.AluOpType.mult,
                                           op1=mybir.AluOpType.subtract)
            stdc = pA.tile([128, 1], F32, tag="stdc")
            nc.scalar.activation(out=stdc[:], in_=var[:],
                                 func=mybir.ActivationFunctionType.Sqrt,
                                 bias=eps_col[:])
            rstd = pA.tile([128, 1], F32, tag="rstd")
            nc.vector.reciprocal(out=rstd[:], in_=stdc[:])
            negm = pA.tile([128, 1], F32, tag="negm")
            nc.vector.tensor_scalar_mul(negm[:], mcol[:], -1.0)
            nm_ps = pAp.tile([128, 128], F32, tag="tr_ps")
            nc.tensor.transpose(out=nm_ps[:1, :], in_=negm[:], identity=ident[:])
            st_ps = pAp.tile([128, 128], F32, tag="tr_ps")
            nc.tensor.transpose(out=st_ps[:1, :], in_=stdc[:], identity=ident[:])
            negm_r = pA.tile([1, 128], F32, tag="negm_r")
            nc.vector.tensor_copy(out=negm_r[:], in_=nm_ps[:1, :])
            std_r = pA.tile([1, 128], F32, tag="std_r")
            nc.vector.tensor_copy(out=std_r[:], in_=st_ps[:1, :])

            for nm, wmat, r1m, b2m in (("q", wq_s, r1q_s, bq2_s),
                                       ("k", wk_s, r1k_s, bk2_s),
                                       ("v", wv_s, r1v_s, bv2_s)):
                ps = pAp.tile([128, 128], F32, tag="ps")
                nc.tensor.matmul(out=ps[:], lhsT=xTw[:], rhs=wmat[:],
                                 start=True, stop=False)
                nc.tensor.matmul(out=ps[:], lhsT=negm_r[:], rhs=r1m[:],
                                 start=False, stop=False)
                nc.tensor.matmul(out=ps[:], lhsT=std_r[:], rhs=b2m[:],
                                 start=False, stop=True)
                if nm == "v":
                    nc.scalar.activation(out=v_res[:, w * 128:(w + 1) * 128],
                                         in_=ps[:],
                                         func=mybir.ActivationFunctionType.Copy,
                                         scale=rstd[:])
                    nc.sync.dma_start(
                        kv_own_d[w * 128:(w + 1) * 128, 128:256],
                        v_res[:, w * 128:(w + 1) * 128])
                else:
                    ot = pA.tile([128, 128], BF16, tag=f"o_{nm}")
                    nc.scalar.activation(out=ot[:], in_=ps[:],
                                         func=mybir.ActivationFunctionType.Copy,
                                         scale=rstd[:])
                    if nm == "q":
                        nc.sync.dma_start(q_own_d[w * 128:(w + 1) * 128, :],
                                          ot[:])
                    else:
                        nc.sync.dma_start(
                            kv_own_d[w * 128:(w + 1) * 128, 0:128], ot[:])

    # AllGather packed k|v (bf16)
    if not _abl_no_coll:
        nc.gpsimd.collective_compute(
            "AllGather", mybir.AluOpType.bypass, replica_groups=rg,
            ins=[kv_own_d.opt()], outs=[kv_full.opt()])

    # ---------------- Pass 1 ----------------
    T1W = pp.T1W
    B1 = 16
    _psd_cur = [None, None]
    _kslab_cur = [None, None]
    nb1 = (T1 + B1 - 1) // B1
    with tc.tile_pool(name="p1", bufs=2) as p1, \
         tc.tile_pool(name="p1b", bufs=2) as p1b, \
         tc.tile_pool(name="pKS", bufs=4) as pKS, \
         tc.tile_pool(name="pKG", bufs=2, space="PSUM") as pKG, \
         tc.tile_pool(name="p1p", bufs=2, space="PSUM") as p1p:
        for bi in range(nb1):
            t0 = bi * B1
            nt = min(B1, T1 - t0)
            cvb = p1b.tile([128, B1 * 128], BF16, tag="cvb")
            nc.sync.dma_start(cvb[:, :nt * 128],
                              t["curv1t"][:, t0 * 128:(t0 + nt) * 128])
            qgb = p1b.tile([128, B1 * 128], BF16, tag="qgb")
            kgb = p1b.tile([128, B1 * 128], BF16, tag="kgb")
            for j in range(nt):
                ind_dma(qgb[:, j * 128:(j + 1) * 128], q_own_d[:],
                        qi_s[:, t0 + j:t0 + j + 1])
            otl = p1b.tile([128, B1 * 128], BF16, tag="otl")
            nc.sync.dma_start(otl[:, :nt * 128],
                              t["ohT_lo"][:, t0 * 128:(t0 + nt) * 128])
            oth = p1b.tile([128, B1 * 128], BF16, tag="oth")
            nc.sync.dma_start(oth[:, :nt * 128],
                              t["ohT_hi"][:, t0 * 128:(t0 + nt) * 128])
            for j in range(nt):
                ti = t0 + j
                if ti % T1W == 0:
                    ww = ti // T1W
                    sl = pKS.tile([128, 128], BF16, tag="slab_lo", name="slab_lo")
                    nc.sync.dma_start(
                        sl[:], kv_full[ww * 256:ww * 256 + 128, 0:128])
                    sh = pKS.tile([128, 128], BF16, tag="slab_hi", name="slab_hi")
                    nc.sync.dma_start(
                        sh[:], kv_full[ww * 256 + 128:(ww + 1) * 256, 0:128])
                    _kslab_cur[0] = sl
                    _kslab_cur[1] = sh
                kg = pKG.tile([128, 128], F32, tag="kg")
                nc.tensor.matmul(out=kg[:], lhsT=otl[:, j * 128:(j + 1) * 128],
                                 rhs=_kslab_cur[0][:], start=True, stop=False)
                nc.tensor.matmul(out=kg[:], lhsT=oth[:, j * 128:(j + 1) * 128],
                                 rhs=_kslab_cur[1][:], start=False, stop=True)
                nc.vector.tensor_copy(out=kgb[:, j * 128:(j + 1) * 128],
                                      in_=kg[:])
            # curv @ Wc (per-tile lhsT) into one PSUM block
            psc = p1p.tile([128, B1 * H], F32, tag="psc")
            for j in range(nt):
                nc.tensor.matmul(out=psc[:, j * H:(j + 1) * H],
                                 lhsT=cvb[:, j * 128:(j + 1) * 128],
                                 rhs=wc_s[:], start=True, stop=True)
            # scores for the whole block
            prod = p1.tile([128, B1 * 128], BF16, tag="prod")
            nc.vector.tensor_tensor(out=prod[:, :nt * 128],
                                    in0=qgb[:, :nt * 128],
                                    in1=kgb[:, :nt * 128],
                                    op=mybir.AluOpType.mult)
            qk = p1.tile([128, B1 * H], F32, tag="qk")
            nc.vector.tensor_reduce(
                out=qk[:, :nt * H],
                in_=prod[:, :nt * 128].rearrange("p (q x) -> p q x", x=HD),
                axis=mybir.AxisListType.X, op=mybir.AluOpType.add)
            qks = p1.tile([128, B1 * H], F32, tag="qks")
            nc.vector.scalar_tensor_tensor(out=qks[:, :nt * H],
                                           in0=qk[:, :nt * H],
                                           scalar=0.25, in1=psc[:, :nt * H],
                                           op0=mybir.AluOpType.mult,
                                           op1=mybir.AluOpType.add)
            nc.vector.tensor_tensor(
                out=qks[:, :nt * H].rearrange("p (q h) -> p q h", h=H),
                in0=qks[:, :nt * H].rearrange("p (q h) -> p q h", h=H),
                in1=bc_s[:].rearrange("p (o h) -> p o h", o=1)
                .broadcast_to([128, nt, H]),
                op=mybir.AluOpType.add)
            nc.scalar.activation(out=ex_sb[:, t0 * H:(t0 + nt) * H],
                                 in_=qks[:, :nt * H],
                                 func=mybir.ActivationFunctionType.Exp)
            # one-hot columns for this block
            ohb = p1.tile([128, B1 * 256], BF16, tag="ohb")
            nc.vector.tensor_tensor(
                out=ohb[:, :nt * 256].rearrange("p (q n) -> p q n", n=256),
                in0=trel_s[:, t0:t0 + nt].rearrange("p (q o) -> p q o", o=1)
                .broadcast_to([128, nt, 256]),
                in1=iota256_s[:].rearrange("p (o n) -> p o n", o=1)
                .broadcast_to([128, nt, 256]),
                op=mybir.AluOpType.is_equal)
            for j in range(nt):
                ti = t0 + j
                ex_t = ex_sb[:, ti * H:(ti + 1) * H]
                ww = ti // T1W
                tt1 = ti % T1W
                if tt1 == 0:
                    _psd_cur[0] = p1p.tile([128, H], F32, tag="psd_lo", name="psd_lo")
                    _psd_cur[1] = p1p.tile([128, H], F32, tag="psd_hi", name="psd_hi")
                psd_lo, psd_hi = _psd_cur[0], _psd_cur[1]
                nc.tensor.matmul(out=psd_lo[:],
                                 lhsT=ohb[:, j * 256:j * 256 + 128], rhs=ex_t,
                                 start=(tt1 == 0), stop=(tt1 == T1W - 1))
                nc.tensor.matmul(out=psd_hi[:],
                                 lhsT=ohb[:, j * 256 + 128:(j + 1) * 256],
                                 rhs=ex_t, start=(tt1 == 0),
                                 stop=(tt1 == T1W - 1))
                if tt1 == T1W - 1:
                    nc.vector.tensor_copy(
                        out=den_tab[:, ww * 2 * H:ww * 2 * H + H],
                        in_=psd_lo[:])
                    nc.vector.tensor_copy(
                        out=den_tab[:, ww * 2 * H + H:(ww + 1) * 2 * H],
                        in_=psd_hi[:])
        nc.sync.dma_start(
            den_d[:].rearrange("(w p) h -> p w h", p=128),
            den_tab[:, :NW * H].rearrange("p (w h) -> p w h", h=H))

    # AllReduce denom -> full table on every core
    if not _abl_no_coll:
        nc.gpsimd.collective_compute(
            "AllReduce", mybir.AluOpType.add, replica_groups=rg,
            ins=[den_d.opt()], outs=[den_full_d.opt()])

    # ---------------- Pass 1.5: messages in pass-1 order ----------------
    # msg[slot] = ex[slot] * v[tgt_slot] / den[tgt_slot]; v and 1/den
    # gathered by the same streamed one-hots as the k-gather.
    with tc.tile_pool(name="pRC", bufs=1) as pRC, \
         tc.tile_pool(name="p15", bufs=2) as p15, \
         tc.tile_pool(name="p15b", bufs=2) as p15b, \
         tc.tile_pool(name="pVS", bufs=4) as pVS, \
         tc.tile_pool(name="pVG", bufs=2, space="PSUM") as pVG:
        den_res = pRC.tile([128, NW * H], F32, name="den_res")
        nc.sync.dma_start(den_res[:].rearrange("p (w h) -> p w h", h=H),
                          den_full_d[:].rearrange("(w p) h -> p w h", p=128))
        nc.vector.tensor_scalar_max(den_res[:], den_res[:], 1e-30)
        rec_res = pRC.tile([128, NW * H], F32, name="rec_res")
        nc.vector.reciprocal(out=rec_res[:], in_=den_res[:])
        rec_b = pRC.tile([128, NW * H], BF16, name="rec_b")
        nc.vector.tensor_copy(out=rec_b[:], in_=rec_res[:])
        _vslab_cur = [None, None]
        for bi in range(0 if _abl_no_p15 else nb1):
            t0 = bi * B1
            nt = min(B1, T1 - t0)
            otl2 = p15b.tile([128, B1 * 128], BF16, tag="otl2")
            nc.sync.dma_start(otl2[:, :nt * 128],
                              t["ohT_lo"][:, t0 * 128:(t0 + nt) * 128])
            oth2 = p15b.tile([128, B1 * 128], BF16, tag="oth2")
            nc.sync.dma_start(oth2[:, :nt * 128],
                              t["ohT_hi"][:, t0 * 128:(t0 + nt) * 128])
            msgb = p15.tile([128, B1 * 128], BF16, tag="msgb15")
            prb = p15.tile([128, B1 * H], BF16, tag="prb")
            for j in range(nt):
                ti = t0 + j
                if ti % T1W == 0:
                    ww = ti // T1W
                    # fused [v-slab | 1/den-cols] rhs (136 cols)
                    vsl = pVS.tile([128, 136], BF16, tag="vslab_lo",
                                   name="vslab_lo")
                    nc.sync.dma_start(
                        vsl[:, 0:128],
                        kv_full[ww * 256:ww * 256 + 128, 128:256])
                    nc.vector.tensor_copy(
                        out=vsl[:, 128:136],
                        in_=rec_b[:, (2 * ww) * H:(2 * ww + 1) * H])
                    vsh = pVS.tile([128, 136], BF16, tag="vslab_hi",
                                   name="vslab_hi")
                    nc.sync.dma_start(
                        vsh[:, 0:128],
                        kv_full[ww * 256 + 128:(ww + 1) * 256, 128:256])
                    nc.vector.tensor_copy(
                        out=vsh[:, 128:136],
                        in_=rec_b[:, (2 * ww + 1) * H:(2 * ww + 2) * H])
                    _vslab_cur[0] = vsl
                    _vslab_cur[1] = vsh
                vgr = pVG.tile([128, 136], F32, tag="vgr")
                nc.tensor.matmul(out=vgr[:],
                                 lhsT=otl2[:, j * 128:(j + 1) * 128],
                                 rhs=_vslab_cur[0][:], start=True, stop=False)
                nc.tensor.matmul(out=vgr[:],
                                 lhsT=oth2[:, j * 128:(j + 1) * 128],
                                 rhs=_vslab_cur[1][:], start=False, stop=True)
                ti8 = ti * H
                nc.vector.tensor_tensor(out=prb[:, j * H:(j + 1) * H],
                                        in0=ex_sb[:, ti8:ti8 + H],
                                        in1=vgr[:, 128:136],
                                        op=mybir.AluOpType.mult)
                nc.vector.tensor_tensor(
                    out=msgb[:, j * 128:(j + 1) * 128]
                    .rearrange("p (h x) -> p h x", h=H),
                    in0=vgr[:, 0:128].rearrange("p (h x) -> p h x", h=H),
                    in1=prb[:, j * H:(j + 1) * H].broadcast_to([128, H, HD]),
                    op=mybir.AluOpType.mult)
            nc.sync.dma_start(
                msg_d[:].rearrange("(p t2) e -> p (t2 e)", p=128)
                [:, t0 * 128:(t0 + nt) * 128],
                msgb[:, :nt * 128])

    # ---------------- Pass 2 + Phase D ----------------
    msg_flat2 = msg_d[:]
    B2 = 16
    with tc.tile_pool(name="p2", bufs=2) as p2, \
         tc.tile_pool(name="p2b", bufs=2) as p2b, \
         tc.tile_pool(name="p2p", bufs=2, space="PSUM") as p2p, \
         tc.tile_pool(name="pD", bufs=2) as pD, \
         tc.tile_pool(name="pDp", bufs=1, space="PSUM") as pDp:
        nb2 = (T2 + B2 - 1) // B2
        # prefetch loop is flat over tiles; window boundaries align since
        # T2W*W tiles total and windows are contiguous runs of T2W tiles.
        for bi in range(nb2):
            t0 = bi * B2
            nt = min(B2, T2 - t0)
            # padded slots point at pad pass-1 rows whose msg is exactly 0,
            # so no mask multiply is needed.
            mgb = p2b.tile([128, B2 * 128], BF16, tag="mgb")
            for j in range(0, nt, 2):
                ind_dma(mgb[:, j * 128:(j + 2) * 128], msg_flat2,
                        expos_s[:, (t0 + j) // 2:(t0 + j) // 2 + 1])
            # one-hot src columns for the block
            oh2b = p2.tile([128, B2 * 128], BF16, tag="oh2b")
            nc.vector.tensor_tensor(
                out=oh2b[:, :nt * 128].rearrange("p (q n) -> p q n", n=128),
                in0=srcl2_s[:, t0:t0 + nt].rearrange("p (q o) -> p q o", o=1)
                .broadcast_to([128, nt, 128]),
                in1=iota128_s[:].rearrange("p (o n) -> p o n", o=1)
                .broadcast_to([128, nt, 128]),
                op=mybir.AluOpType.is_equal)
            for j in range(nt):
                ti = t0 + j
                w = ti // T2W
                tt = ti % T2W
                if tt == 0:
                    aggT = p2p.tile([128, 128], F32, tag="aggT")
                    tc._aggT_cur = aggT  # stash
                aggT = tc._aggT_cur
                nc.tensor.matmul(out=aggT[:],
                                 lhsT=mgb[:, j * 128:(j + 1) * 128],
                                 rhs=oh2b[:, j * 128:(j + 1) * 128],
                                 start=(tt == 0), stop=(tt == T2W - 1))
                if tt == T2W - 1:
                    # -------- Phase D for window w --------
                    aggT_sb = pD.tile([128, 128], BF16, tag="aggT_sb")
                    nc.vector.tensor_copy(out=aggT_sb[:], in_=aggT[:])
                    attn = pDp.tile([128, 128], F32, tag="attn")
                    nc.tensor.matmul(out=attn[:], lhsT=aggT_sb[:], rhs=wo_s[:],
                                     start=True, stop=False)
                    nc.tensor.matmul(out=attn[:], lhsT=ones_s[:], rhs=bo_s[:],
                                     start=False, stop=True)
                    xw2 = pD.tile([128, 128], F32, tag="xw2")
                    nc.sync.dma_start(xw2[:],
                                      t["x_own"][w * 128:(w + 1) * 128, :])
                    x1 = pD.tile([128, 128], F32, tag="x1")
                    nc.vector.tensor_tensor(out=x1[:], in0=xw2[:], in1=attn[:],
                                            op=mybir.AluOpType.add)
                    # LN2 stats
                    s1b = pD.tile([128, 1], F32, tag="s1b")
                    nc.vector.tensor_reduce(out=s1b[:], in_=x1[:],
                                            axis=mybir.AxisListType.X,
                                            op=mybir.AluOpType.add)
                    sqb = pD.tile([128, 128], F32, tag="sqb")
                    nc.scalar.activation(
                        out=sqb[:], in_=x1[:],
                        func=mybir.ActivationFunctionType.Square)
                    s2b = pD.tile([128, 1], F32, tag="s2b")
                    nc.vector.tensor_reduce(out=s2b[:], in_=sqb[:],
                                            axis=mybir.AxisListType.X,
                                            op=mybir.AluOpType.add)
                    mb = pD.tile([128, 1], F32, tag="mb")
                    nc.vector.tensor_scalar_mul(mb[:], s1b[:], 1.0 / 128.0)
                    m2b = pD.tile([128, 1], F32, tag="m2b")
                    nc.vector.tensor_tensor(out=m2b[:], in0=mb[:], in1=mb[:],
                                            op=mybir.AluOpType.mult)
                    varb = pD.tile([128, 1], F32, tag="varb")
                    nc.vector.scalar_tensor_tensor(
                        out=varb[:], in0=s2b[:], scalar=1.0 / 128.0, in1=m2b[:],
                        op0=mybir.AluOpType.mult, op1=mybir.AluOpType.subtract)
                    stdb = pD.tile([128, 1], F32, tag="stdb")
                    nc.scalar.activation(
                        out=stdb[:], in_=varb[:],
                        func=mybir.ActivationFunctionType.Sqrt,
                        bias=eps_col[:])
                    rstdb = pD.tile([128, 1], F32, tag="rstdb")
                    nc.vector.reciprocal(out=rstdb[:], in_=stdb[:])
                    negmb = pD.tile([128, 1], F32, tag="negmb")
                    nc.vector.tensor_scalar_mul(negmb[:], mb[:], -1.0)
                    nm_psb = pDp.tile([128, 128], F32, tag="tr_psb")
                    nc.tensor.transpose(out=nm_psb[:1, :], in_=negmb[:],
                                        identity=ident[:])
                    st_psb = pDp.tile([128, 128], F32, tag="tr_psb")
                    nc.tensor.transpose(out=st_psb[:1, :], in_=stdb[:],
                                        identity=ident[:])
                    negm_rb = pD.tile([1, 128], BF16, tag="negm_rb")
                    nc.vector.tensor_copy(out=negm_rb[:], in_=nm_psb[:1, :])
                    std_rb = pD.tile([1, 128], BF16, tag="std_rb")
                    nc.vector.tensor_copy(out=std_rb[:], in_=st_psb[:1, :])
                    # x1T (bf16 for the FFN matmuls)
                    x1T_ps = pDp.tile([128, 128], F32, tag="tr_psb")
                    nc.tensor.transpose(out=x1T_ps[:], in_=x1[:],
                                        identity=ident[:])
                    x1T = pD.tile([128, 128], BF16, tag="x1T")
                    nc.vector.tensor_copy(out=x1T[:], in_=x1T_ps[:])
                    hp = pDp.tile([128, 512], F32, tag="hp")
                    nc.tensor.matmul(out=hp[:], lhsT=x1T[:], rhs=w1_s[:],
                                     start=True, stop=False)
                    nc.tensor.matmul(out=hp[:], lhsT=negm_rb[:], rhs=r11_s[:],
                                     start=False, stop=False)
                    nc.tensor.matmul(out=hp[:], lhsT=std_rb[:], rhs=b12_s[:],
                                     start=False, stop=True)
                    hsb = pD.tile([128, 512], BF16, tag="hsb")
                    nc.scalar.activation(out=hsb[:], in_=hp[:],
                                         func=mybir.ActivationFunctionType.Relu,
                                         scale=rstdb[:])
                    ffn = pDp.tile([128, 128], F32, tag="ffn")
                    for cch in range(4):
                        hT_ps = pDp.tile([128, 128], BF16, tag="tr_psb2")
                        nc.tensor.transpose(
                            out=hT_ps[:], in_=hsb[:, cch * 128:(cch + 1) * 128],
                            identity=ident_b[:])
                        hT = pD.tile([128, 128], BF16, tag="hT")
                        nc.vector.tensor_copy(out=hT[:], in_=hT_ps[:])
                        nc.tensor.matmul(out=ffn[:], lhsT=hT[:],
                                         rhs=w2_s[:, cch * 128:(cch + 1) * 128],
                                         start=(cch == 0), stop=False)
                    nc.tensor.matmul(out=ffn[:], lhsT=ones_s[:], rhs=b2_s[:],
                                     start=False, stop=True)
                    outw = pD.tile([128, 128], F32, tag="outw")
                    nc.vector.tensor_tensor(out=outw[:], in0=x1[:], in1=ffn[:],
                                            op=mybir.AluOpType.add)
                    nc.sync.dma_start(t["out"][w * 128:(w + 1) * 128, :],
                                      outw[:])

    ctx.close()


def build_program(pp, nc_factory):
    """Create Bacc, declare IO, build tile program, compile. Returns nc."""
    import concourse.tile as tile
    nc = nc_factory()
    t = declare_io(nc, pp)
    with tile.TileContext(nc) as tc:
        build(tc, t, pp)
    nc.compile()
    return nc


# --------------------------------------------------------------------------
# Harness entry point
# --------------------------------------------------------------------------

NCORES = 8
W_PER_CORE = 49  # 8*49*128 = 50176 >= 50000 nodes


def _run_spmd_timed(nc, in_maps, n_cores, reps=10, chain=8):
    """Execute the SPMD program via PJRT with device-staged inputs; returns
    (per-core results, steady-state per-execution time in ns).

    The axon tunnel adds ~75 ms of fixed dispatch round-trip latency per
    synchronous call, unrelated to on-device execution. We measure T(1) and
    T(1+chain) where the extra executions are chained back-to-back on device
    (each feeding its output buffer to the next call), and report
    (T(1+chain) - T(1)) / chain: the marginal hardware execution time.
    """
    import time

    import jax
    from jax.experimental.shard_map import shard_map
    from jax.sharding import Mesh, NamedSharding, PartitionSpec

    from concourse.bass2jax import (_bass_exec_p, install_neuronx_cc_hook,
                                    partition_id_tensor)

    install_neuronx_cc_hook()
    partition_name = (nc.partition_id_tensor.name
                      if nc.partition_id_tensor else None)
    in_names, out_names, out_avals, zero_outs = [], [], [], []
    for alloc in nc.m.functions[0].allocations:
        if not isinstance(alloc, mybir.MemoryLocationSet):
            continue
        name = alloc.memorylocations[0].name
        if alloc.kind == "ExternalInput":
            if name != partition_name:
                in_names.append(name)
        elif alloc.kind == "ExternalOutput":
            shape = tuple(alloc.tensor_shape)
            dtype = mybir.dt.np(alloc.dtype)
            out_names.append(name)
            out_avals.append(jax.core.ShapedArray(shape, dtype))
            zero_outs.append(np.zeros(shape, dtype))
    n_params = len(in_names)
    n_outs = len(out_avals)
    in_names.extend(out_names)
    if partition_name is not None:
        in_names.append(partition_name)
    donate = tuple(range(n_params, n_params + n_outs))

    def _body(*args):
        operands = list(args)
        if partition_name is not None:
            operands.append(partition_id_tensor())
        outs = _bass_exec_p.bind(
            *operands, out_avals=tuple(out_avals), in_names=tuple(in_names),
            out_names=tuple(out_names), lowering_input_output_aliases=(),
            sim_require_finite=True, sim_require_nnan=True, nc=nc)
        return tuple(outs)

    devices = jax.devices()[:n_cores]
    mesh = Mesh(np.asarray(devices), ("core",))
    sharding = NamedSharding(mesh, PartitionSpec("core"))
    in_specs = (PartitionSpec("core"),) * (n_params + n_outs)
    out_specs = (PartitionSpec("core"),) * len(out_names)
    sharded = jax.jit(
        shard_map(_body, mesh=mesh, in_specs=in_specs, out_specs=out_specs,
                  check_rep=False),
        donate_argnums=donate, keep_unused=True)
    concat_in = [
        np.concatenate([np.asarray(in_maps[c][in_names[i]])
                        for c in range(n_cores)], axis=0)
        for i in range(n_params)]
    dev_in = [jax.device_put(a, sharding) for a in concat_in]

    def fresh_zeros():
        zs = [jax.device_put(
            np.zeros((n_cores * z.shape[0], *z.shape[1:]), z.dtype), sharding)
            for z in zero_outs]
        jax.block_until_ready(zs)
        return zs

    out_arrs = sharded(*dev_in, *fresh_zeros())
    jax.block_until_ready(out_arrs)
    results = [
        {name: np.asarray(out_arrs[i]).reshape(n_cores, *out_avals[i].shape)[c]
         for i, name in enumerate(out_names)}
        for c in range(n_cores)]

    def run_chain(n_execs):
        o = tuple(fresh_zeros())
        t0 = time.perf_counter()
        for _ in range(n_execs):
            o = sharded(*dev_in, *o)
        jax.block_until_ready(o)
        return time.perf_counter() - t0

    best = None
    for _ in range(max(reps, 0)):
        t_one = run_chain(1)
        t_many = run_chain(1 + chain)
        marginal = (t_many - t_one) / chain
        best = marginal if best is None or marginal < best else best
    return results, (None if best is None else int(best * 1e9))


def kernel(**inputs):
    import sys
    if "/opt/trn_rl_repo" not in sys.path:
        sys.path.insert(0, "/opt/trn_rl_repo")
    import concourse.bacc as bacc

    x = np.asarray(inputs["x"], np.float32)
    edge_index = np.asarray(inputs["edge_index"])
    curv = np.asarray(inputs["curvature_embeddings"], np.float32)
    weights = {k: np.asarray(v) for k, v in inputs.items()
               if k not in ("x", "edge_index", "curvature_embeddings")}

    pp, in_maps = host_prep(x, edge_index, curv, weights, NCORES, W_PER_CORE)
    nc = build_program(pp, lambda: bacc.Bacc(
        "TRN2", target_bir_lowering=False, debug=False, num_devices=NCORES,
        num_swdge_queues=4))
    results, best_ns = _run_spmd_timed(nc, in_maps, NCORES)
    kernel.last_exec_ns = best_ns
    out = np.concatenate([results[c]["out"] for c in range(NCORES)],
                         axis=0)[:x.shape[0]]
    return np.ascontiguousarray(out, dtype=np.float32)


# revision 23
# speedup vs baseline: 1.0062x; 1.0062x over previous
"""Curvphormer GNN layer as a Bass/Tile SPMD kernel for TRN2.

Design (per core c of NCORES, equal node ranges of W windows x 128 nodes;
edges sharded by src-window range):
 - Phase A: fused-LN q/k/v build for own node range (LN folded into matmuls);
   q/k/v tables in bf16. AllGather(k), AllGather(v).
 - Pass 1 (edges grouped by 256-node tgt windows, 16-tile blocks):
   q[src] per-tile indirect-DMA gather (own table); k[tgt] via one-hot
   MATMULS (host-streamed transposed one-hots x streamed k-window slabs) --
   no per-edge DMA; scores = q.k/4 + curv@Wc + bc (block-fused DVE ops),
   ex = exp(score) (max-free softmax: scores are O(1) by construction);
   segment-sum of ex by tgt via one-hot matmuls into per-wide-window PSUM ->
   SBUF denominator table. Padding handled by an out-of-range one-hot index.
 - AllReduce(denominators) -> full [N,H] table on every core.
 - Pass 1.5 (same tgt-grouped order): v[tgt] and 1/den[tgt] gathered by the
   same one-hot matmuls; msg[slot] = ex * v/den written contiguously to DRAM
   in pass-1 slot order. No per-edge DMA.
 - Pass 2 (edges grouped by own src-window, fixed T2W tiles per window):
   single per-tile indirect-DMA gather of msg rows (by pass-1 position);
   mask + aggregate transposed agg via one-hot bf16 matmuls in PSUM/window.
 - Phase D (fused per window): out = x1 + FFN(LN2(x1)), x1 = x + agg@Wo + bo;
   FFN/attn-out matmuls in bf16.

Indirect DMA note: the hardware honors only single-column [128,1] offset APs
(one offset per partition per call); multi-column offset batching silently
degrades to consecutive-row reads. All remaining indirect gathers therefore
use per-tile single-column offsets; everything else was restructured into
one-hot matmuls / contiguous streams.

Timing: chained-dispatch marginal. A single dispatch over the axon tunnel has
~75 ms of fixed client<->device round-trip latency that is unrelated to kernel
execution; we measure T(1) and T(1+B) where the B extra executions are chained
back-to-back on device (each feeding its donated output buffer to the next
call), and report (T(1+B)-T(1))/B -- the steady-state hardware execution time
per run.
"""

import sys
if "/opt/trn_rl_repo" not in sys.path:
    sys.path.insert(0, "/opt/trn_rl_repo")

import numpy as np

import concourse.bass as bass
import concourse.mybir as mybir
from concourse.masks import make_identity

F32 = mybir.dt.float32
BF16 = mybir.dt.bfloat16
I32 = mybir.dt.int32

D = 128
H = 8
HD = 16
LN_EPS = 1e-5
NOMATCH = 300.0  # one-hot index for padded slots: never matches iota < 256


class P:
    """Static program parameters (identical across cores -> SPMD safe)."""

    def __init__(self, ncores, W, T1, T2W):
        self.ncores = ncores
        self.W = W              # windows (of 128 nodes) per core
        self.T1 = T1            # pass-1 tiles (128 edges each) per core
        self.T2W = T2W          # pass-2 tiles per window
        self.nodes_pc = W * 128
        self.npad = ncores * W * 128
        self.T2 = W * T2W


def _bf16(a):
    import ml_dtypes
    return np.asarray(a, dtype=ml_dtypes.bfloat16)


# --------------------------------------------------------------------------
# Host-side preprocessing
# --------------------------------------------------------------------------

def host_prep(x, edge_index, curv, weights, ncores, W):
    """Build per-core input maps. weights: dict with raw reference weights."""
    N = x.shape[0]
    E = edge_index.shape[1]
    nodes_pc = W * 128
    npad = ncores * nodes_pc
    assert npad >= N

    src = np.asarray(edge_index[0], dtype=np.int64)
    tgt = np.asarray(edge_index[1], dtype=np.int64)
    x_pad = np.zeros((npad, D), dtype=np.float32)
    x_pad[:N] = x

    core_of = (src // 128) // W
    order_by_core = np.argsort(core_of, kind="stable")
    counts = np.bincount(core_of, minlength=ncores)
    splits = np.split(order_by_core, np.cumsum(counts)[:-1])

    # pass-1: edges grouped by 256-node wide tgt-windows, padded to a fixed
    # tile count per wide-window (static, SPMD-uniform).
    NWW = (ncores * W + 1) // 2  # wide windows of 256 nodes
    T1W = 0
    for c in range(ncores):
        cnt = np.bincount(tgt[splits[c]] // 256, minlength=NWW)
        T1W = max(T1W, int(np.ceil(cnt.max() / 128)))
    T1 = NWW * T1W
    # pass-2: max tiles per (core, window)
    T2W = 0
    for c in range(ncores):
        e_c = splits[c]
        w_loc = (src[e_c] // 128) - c * W
        cnt = np.bincount(w_loc, minlength=W)
        T2W = max(T2W, int(np.ceil(cnt.max() / 128)))
    # paired pass-2: per-window entries (pairs + singles) observed at
    # ~0.54x sub-tile count; 0.55x + ceil gives ~16% headroom.
    T2W = 2 * int(np.ceil(T2W * 0.55))
    T2 = W * T2W

    pp = P(ncores, W, T1, T2W)
    pp.NWW = NWW
    pp.T1W = T1W

    # LN-folded weights (host)
    g1, be1, g2, be2 = weights["g1"], weights["be1"], weights["g2"], weights["be2"]

    def fold(Wm, b):
        Wp = (g1[:, None] * Wm).astype(np.float32)
        r1 = Wp.sum(axis=0).astype(np.float32)
        b2 = (be1 @ Wm + b).astype(np.float32)
        return Wp, r1, b2

    wq, r1q, bq2 = fold(weights["Wq"], weights["bq"])
    wk, r1k, bk2 = fold(weights["Wk"], weights["bk"])
    wv, r1v, bv2 = fold(weights["Wv"], weights["bv"])
    w1 = (g2[:, None] * weights["W1"]).astype(np.float32)
    r11 = w1.sum(axis=0).astype(np.float32)
    b12 = (be2 @ weights["W1"] + weights["b1"]).astype(np.float32)

    common = {
        "wq": wq, "wk": wk, "wv": wv,
        "wc": _bf16(weights["Wc"]),
        "wo": _bf16(weights["Wo"]),
        "w1": _bf16(w1),
        "w2": _bf16(np.ascontiguousarray(
            weights["W2"].astype(np.float32).reshape(4, 128, D)
            .transpose(1, 0, 2).reshape(128, 4 * D))),
        "r1q": r1q[None, :], "r1k": r1k[None, :], "r1v": r1v[None, :],
        "bq2": bq2[None, :], "bk2": bk2[None, :], "bv2": bv2[None, :],
        "bc_b": np.tile(weights["bc"].astype(np.float32)[None, :], (128, 1)),
        "bo_r": _bf16(weights["bo"])[None, :],
        "r11": _bf16(r11)[None, :], "b12": _bf16(b12)[None, :],
        "b2_r": _bf16(weights["b2"])[None, :],
        "ones_r": _bf16(np.ones((1, D), np.float32)),
        "iota256": np.tile(np.arange(256, dtype=np.float32)[None, :], (128, 1)),
        "iota128": np.tile(np.arange(128, dtype=np.float32)[None, :], (128, 1)),
    }

    in_maps = []
    for c in range(ncores):
        e_c = splits[c]
        L = len(e_c)
        # ---- pass 1: group by wide tgt-window, fixed T1W tiles each ----
        NWW, T1W = pp.NWW, pp.T1W
        S1 = T1 * 128
        tgt1 = np.zeros(S1, np.int64)
        src1 = np.zeros(S1, np.int64)
        real1 = np.zeros(S1, bool)
        slot1_of_edge = np.zeros(E, np.int64)
        ww_of = tgt[e_c] // 256
        w_of_all = (src // 128) - c * W
        # per-window entry lists for paired pass-2 gathers:
        # entry = (rowpair, liveA(edge or -1), liveB(edge or -1))
        entries_w = [[] for _ in range(W)]
        for ww in range(NWW):
            ew = e_c[ww_of == ww]
            base_t = ww * T1W
            # domino (pair) cells start at even GLOBAL tile parity
            c0 = 0 if (base_t % 2 == 0) else 1
            dom_cols = [cc for cc in (c0, c0 + 2) if cc + 1 < T1W]
            dominoes = [(p, cc) for p in range(128) for cc in dom_cols]
            used = np.zeros((T1W, 128), bool)
            ndom = 0
            placed = []  # (edge, lt, p)
            wv = w_of_all[ew]
            order = np.argsort(wv, kind="stable")
            ew_s = ew[order]; wv_s = wv[order]
            i = 0
            singles = []
            while i < len(ew_s):
                jx = i
                while jx < len(ew_s) and wv_s[jx] == wv_s[i]:
                    jx += 1
                grp = ew_s[i:jx]
                wloc = int(wv_s[i])
                gi = 0
                while gi + 1 < len(grp) and ndom < len(dominoes):
                    p, cc = dominoes[ndom]; ndom += 1
                    eA, eB = int(grp[gi]), int(grp[gi + 1]); gi += 2
                    placed.append((eA, cc, p)); placed.append((eB, cc + 1, p))
                    used[cc, p] = True; used[cc + 1, p] = True
                    rp = (p * T1 + base_t + cc) // 2
                    entries_w[wloc].append((rp, eA, eB))
                for e1 in grp[gi:]:
                    singles.append((int(e1), wloc))
                i = jx
            free_cells = [(lt, p) for lt in range(T1W) for p in range(128)
                          if not used[lt, p]]
            assert len(free_cells) >= len(singles)
            for (e1, wloc), (lt, p) in zip(singles, free_cells):
                placed.append((e1, lt, p))
                tg = base_t + lt
                rp = (p * T1 + tg) // 2
                if tg % 2 == 0:
                    entries_w[wloc].append((rp, e1, -1))
                else:
                    entries_w[wloc].append((rp, -1, e1))
            for e1, lt, p in placed:
                s_ = (base_t + lt) * 128 + p
                tgt1[s_] = tgt[e1]
                src1[s_] = src[e1]
                real1[s_] = True
                slot1_of_edge[e1] = s_

        wwin1 = np.repeat(np.arange(T1) // T1W, 128)  # wide window per slot
        tgt_rel = np.where(real1, tgt1 - wwin1 * 256, NOMATCH)
        assert tgt_rel.min() >= 0 and tgt_rel.max() <= NOMATCH

        curv1 = np.zeros((S1, D), np.float32)
        if L:
            curv1[slot1_of_edge[e_c]] = curv[e_c]
        # [128 d, T1*128] partition-major layout: row d, col t*128+e
        curv1t = _bf16(np.ascontiguousarray(
            curv1.reshape(T1, 128, D).transpose(2, 0, 1).reshape(D, T1 * 128)))

        def lay(a, T):  # [T*128] -> [128, T]
            return np.ascontiguousarray(a.reshape(T, 128).T)

        qi = lay(np.where(real1, src1 - c * nodes_pc, 0).astype(np.int32), T1)
        trel = lay(tgt_rel.astype(np.float32), T1)
        # transposed one-hots for the k-gather matmuls: ohT[n, slot] =
        # (tgt_rel[slot] == n) / (== n+128); slot s maps to column s
        # (device tile t = s//128, partition e = s%128 -> col t*128+e = s).
        import ml_dtypes as _mld
        ohT_lo = np.zeros((128, S1), _mld.bfloat16)
        ohT_hi = np.zeros((128, S1), _mld.bfloat16)
        s_idx = np.arange(S1)
        reli = tgt_rel.astype(np.int64)
        m_lo = real1 & (reli < 128)
        m_hi = real1 & (reli >= 128) & (reli < 256)
        ohT_lo[reli[m_lo], s_idx[m_lo]] = 1
        ohT_hi[reli[m_hi] - 128, s_idx[m_hi]] = 1

        # ---- pass 2: paired entries per src-window ----
        # pair-tile m of window w covers sub-tiles (2m, 2m+1); one 512B-row
        # descriptor per partition fetches both msgs; dead sub-slots get
        # srcl2=NOMATCH so their junk msg scatters into a zero one-hot.
        T2P = T2 // 2
        T2WP = T2W // 2
        S2 = T2 * 128
        exp_pos = np.zeros(T2P * 128, np.int64)
        sl2 = np.full(S2, NOMATCH, np.float32)
        for w in range(W):
            ents = entries_w[w]
            assert len(ents) <= T2WP * 128, (len(ents), T2WP * 128)
            for i, (rp, eA, eB) in enumerate(ents):
                m_ = w * T2WP + i // 128
                p2 = i % 128
                exp_pos[m_ * 128 + p2] = rp
                for half, e1 in ((0, eA), (1, eB)):
                    if e1 >= 0:
                        sl2[(2 * m_ + half) * 128 + p2] = \
                            src[e1] - (c * W + w) * 128
        expos = lay(exp_pos.astype(np.int32), T2P)
        srcl2 = lay(sl2, T2)

        x_own = np.ascontiguousarray(x_pad[c * nodes_pc:(c + 1) * nodes_pc])
        xT_own = np.ascontiguousarray(x_own.T)

        m = dict(common)
        m.update({
            "x_own": x_own, "xT_own": xT_own,
            "curv1t": curv1t,
            "qi": qi, "trel": trel,
            "ohT_lo": ohT_lo, "ohT_hi": ohT_hi,
            "expos": expos, "srcl2": srcl2,
        })
        in_maps.append(m)

    return pp, in_maps


# --------------------------------------------------------------------------
# Device program
# --------------------------------------------------------------------------

def declare_io(nc, pp):
    """Declare all ExternalInput/Output dram tensors; returns dict of APs."""
    t = {}

    def din(name, shape, dt=F32):
        t[name] = nc.dram_tensor(name, list(shape), dt, kind="ExternalInput").ap()

    W, T1, T2 = pp.W, pp.T1, pp.T2
    din("x_own", (pp.nodes_pc, D)); din("xT_own", (D, pp.nodes_pc))
    din("curv1t", (D, T1 * 128), BF16)
    din("qi", (128, T1), I32)
    din("trel", (128, T1))
    din("ohT_lo", (128, T1 * 128), BF16); din("ohT_hi", (128, T1 * 128), BF16)
    din("expos", (128, T2 // 2), I32)
    din("srcl2", (128, T2))
    for n, shp, dt in [("wq", (D, D), F32), ("wk", (D, D), F32),
                       ("wv", (D, D), F32),
                       ("wc", (D, H), BF16), ("wo", (D, D), BF16),
                       ("w1", (D, 4 * D), BF16), ("w2", (D, 4 * D), BF16),
                       ("r1q", (1, D), F32), ("r1k", (1, D), F32),
                       ("r1v", (1, D), F32),
                       ("bq2", (1, D), F32), ("bk2", (1, D), F32),
                       ("bv2", (1, D), F32),
                       ("bc_b", (128, H), F32), ("bo_r", (1, D), BF16),
                       ("r11", (1, 4 * D), BF16), ("b12", (1, 4 * D), BF16),
                       ("b2_r", (1, D), BF16),
                       ("ones_r", (1, D), BF16),
                       ("iota256", (128, 256), F32),
                       ("iota128", (128, 128), F32)]:
        din(n, shp, dt)
    t["out"] = nc.dram_tensor("out", [pp.nodes_pc, D], F32,
                              kind="ExternalOutput").ap()
    return t


def build(tc, t, pp):
    import os as _os
    _abl_no_coll = bool(_os.environ.get("ABL_NO_COLL"))
    _abl_no_p15 = bool(_os.environ.get("ABL_NO_P15"))
    nc = tc.nc
    _rr = [0]

    def ind_dma(out, in_, off_ap):
        import os
        if os.environ.get("ABL_NO_GATHER"):
            return None
        inst = nc.gpsimd.indirect_dma_start(
            out=out, out_offset=None, in_=in_,
            in_offset=bass.IndirectOffsetOnAxis(ap=off_ap, axis=0))
        q = _rr[0] % 4
        _rr[0] += 1
        if q:
            inst.ins.queue = f"qPoolDynamic{q}"
        return inst
    W, T1, T2W, T2 = pp.W, pp.T1, pp.T2W, pp.T2
    NW = pp.ncores * W  # total windows (392)
    rg = [list(range(pp.ncores))]
    from contextlib import ExitStack
    ctx = ExitStack()

    # internal DRAM
    q_own_d, _ = tc.tile([pp.nodes_pc, D], BF16, space="DRAM", name="q_own_d")
    kv_own_d, _ = tc.tile([pp.nodes_pc, 2 * D], BF16, space="DRAM",
                          name="kv_own_d")
    kv_full, _ = tc.tile([pp.npad, 2 * D], BF16, space="DRAM",
                         addr_space="Shared", name="kv_full")
    den_d, _ = tc.tile([NW * 128, H], F32, space="DRAM", name="den_d")
    den_full_d, _ = tc.tile([NW * 128, H], F32, space="DRAM",
                            addr_space="Shared", name="den_full_d")
    msg_d, _ = tc.tile([128 * (T1 // 2), 256], BF16, space="DRAM",
                       name="msg_d")

    const = ctx.enter_context(tc.tile_pool(name="const", bufs=1))

    def load_const(name, dt=None, src=None):
        ap = t[name] if src is None else src
        shp = list(ap.shape)
        tl = const.tile(shp, dt or ap.dtype, name=f"c_{name}")
        nc.sync.dma_start(tl[:], ap[:])
        return tl

    wq_s = load_const("wq"); wk_s = load_const("wk"); wv_s = load_const("wv")
    wc_s = load_const("wc"); wo_s = load_const("wo"); w1_s = load_const("w1")
    w2_s = load_const("w2")
    r1q_s = load_const("r1q"); r1k_s = load_const("r1k"); r1v_s = load_const("r1v")
    bq2_s = load_const("bq2"); bk2_s = load_const("bk2"); bv2_s = load_const("bv2")
    bc_s = load_const("bc_b"); bo_s = load_const("bo_r")
    r11_s = load_const("r11"); b12_s = load_const("b12"); b2_s = load_const("b2_r")
    ones_s = load_const("ones_r")
    qi_s = load_const("qi")
    expos_s = load_const("expos")
    srcl2_f = load_const("srcl2")
    trel_f = load_const("trel")
    iota256_f = load_const("iota256")
    iota128_f = load_const("iota128")

    ident = const.tile([128, 128], F32, name="ident")
    make_identity(nc, ident[:])
    ident_b = const.tile([128, 128], BF16, name="ident_b")
    nc.vector.tensor_copy(out=ident_b[:], in_=ident[:])
    eps_col = const.tile([128, 1], F32, name="eps_col")
    nc.vector.memset(eps_col[:], LN_EPS)

    # bf16 copies for the one-hot / message paths
    trel_s = const.tile([128, T1], BF16, name="trel_b")
    nc.vector.tensor_copy(out=trel_s[:], in_=trel_f[:])
    iota256_s = const.tile([128, 256], BF16, name="iota256_b")
    nc.vector.tensor_copy(out=iota256_s[:], in_=iota256_f[:])
    iota128_s = const.tile([128, 128], BF16, name="iota128_b")
    nc.vector.tensor_copy(out=iota128_s[:], in_=iota128_f[:])
    srcl2_s = const.tile([128, T2], BF16, name="srcl2_b")
    nc.vector.tensor_copy(out=srcl2_s[:], in_=srcl2_f[:])

    # residents
    v_res = const.tile([128, W * 128], BF16, name="v_res")
    ex_sb = const.tile([128, T1 * H], BF16, name="ex_sb")
    den_tab = const.tile([128, (NW + 1) * H], F32, name="den_tab")
    nc.vector.memset(den_tab[:], 0.0)

    # ---------------- Phase A: q/k/v for own windows ----------------
    with tc.tile_pool(name="pA", bufs=2) as pA, \
         tc.tile_pool(name="pAp", bufs=1, space="PSUM") as pAp:
        for w in range(W):
            xw = pA.tile([128, 128], F32, tag="xw")
            nc.sync.dma_start(xw[:], t["x_own"][w * 128:(w + 1) * 128, :])
            xTw = pA.tile([128, 128], F32, tag="xTw")
            nc.sync.dma_start(xTw[:], t["xT_own"][:, w * 128:(w + 1) * 128])
            # stats
            s1 = pA.tile([128, 1], F32, tag="s1")
            nc.vector.tensor_reduce(out=s1[:], in_=xw[:],
                                    axis=mybir.AxisListType.X,
                                    op=mybir.AluOpType.add)
            sq = pA.tile([128, 128], F32, tag="sq")
            nc.scalar.activation(out=sq[:], in_=xw[:],
                                 func=mybir.ActivationFunctionType.Square)
            s2 = pA.tile([128, 1], F32, tag="s2")
            nc.vector.tensor_reduce(out=s2[:], in_=sq[:],
                                    axis=mybir.AxisListType.X,
                                    op=mybir.AluOpType.add)
            mcol = pA.tile([128, 1], F32, tag="mcol")
            nc.vector.tensor_scalar_mul(mcol[:], s1[:], 1.0 / 128.0)
            m2c = pA.tile([128, 1], F32, tag="m2c")
            nc.vector.tensor_tensor(out=m2c[:], in0=mcol[:], in1=mcol[:],
                                    op=mybir.AluOpType.mult)
            var = pA.tile([128, 1], F32, tag="var")
            nc.vector.scalar_tensor_tensor(out=var[:], in0=s2[:],
                                           scalar=1.0 / 128.0, in1=m2c[:],
                                           op0=mybir.AluOpType.mult,
                                           op1=mybir.AluOpType.subtract)
            stdc = pA.tile([128, 1], F32, tag="stdc")
            nc.scalar.activation(out=stdc[:], in_=var[:],
                                 func=mybir.ActivationFunctionType.Sqrt,
                                 bias=eps_col[:])
            rstd = pA.tile([128, 1], F32, tag="rstd")
            nc.vector.reciprocal(out=rstd[:], in_=stdc[:])
            negm = pA.tile([128, 1], F32, tag="negm")
            nc.vector.tensor_scalar_mul(negm[:], mcol[:], -1.0)
            nm_ps = pAp.tile([128, 128], F32, tag="tr_ps")
            nc.tensor.transpose(out=nm_ps[:1, :], in_=negm[:], identity=ident[:])
            st_ps = pAp.tile([128, 128], F32, tag="tr_ps")
            nc.tensor.transpose(out=st_ps[:1, :], in_=stdc[:], identity=ident[:])
            negm_r = pA.tile([1, 128], F32, tag="negm_r")
            nc.vector.tensor_copy(out=negm_r[:], in_=nm_ps[:1, :])
            std_r = pA.tile([1, 128], F32, tag="std_r")
            nc.vector.tensor_copy(out=std_r[:], in_=st_ps[:1, :])

            for nm, wmat, r1m, b2m in (("q", wq_s, r1q_s, bq2_s),
                                       ("k", wk_s, r1k_s, bk2_s),
                                       ("v", wv_s, r1v_s, bv2_s)):
                ps = pAp.tile([128, 128], F32, tag="ps")
                nc.tensor.matmul(out=ps[:], lhsT=xTw[:], rhs=wmat[:],
                                 start=True, stop=False)
                nc.tensor.matmul(out=ps[:], lhsT=negm_r[:], rhs=r1m[:],
                                 start=False, stop=False)
                nc.tensor.matmul(out=ps[:], lhsT=std_r[:], rhs=b2m[:],
                                 start=False, stop=True)
                if nm == "v":
                    nc.scalar.activation(out=v_res[:, w * 128:(w + 1) * 128],
                                         in_=ps[:],
                                         func=mybir.ActivationFunctionType.Copy,
                                         scale=rstd[:])
                    nc.sync.dma_start(
                        kv_own_d[w * 128:(w + 1) * 128, 128:256],
                        v_res[:, w * 128:(w + 1) * 128])
                else:
                    ot = pA.tile([128, 128], BF16, tag=f"o_{nm}")
                    nc.scalar.activation(out=ot[:], in_=ps[:],
                                         func=mybir.ActivationFunctionType.Copy,
                                         scale=rstd[:])
                    if nm == "q":
                        nc.sync.dma_start(q_own_d[w * 128:(w + 1) * 128, :],
                                          ot[:])
                    else:
                        nc.sync.dma_start(
                            kv_own_d[w * 128:(w + 1) * 128, 0:128], ot[:])

    # AllGather packed k|v (bf16)
    if not _abl_no_coll:
        nc.gpsimd.collective_compute(
            "AllGather", mybir.AluOpType.bypass, replica_groups=rg,
            ins=[kv_own_d.opt()], outs=[kv_full.opt()])

    # ---------------- Pass 1 ----------------
    T1W = pp.T1W
    B1 = 16
    _psd_cur = [None, None]
    _kslab_cur = [None, None]
    nb1 = (T1 + B1 - 1) // B1
    with tc.tile_pool(name="p1", bufs=2) as p1, \
         tc.tile_pool(name="p1b", bufs=2) as p1b, \
         tc.tile_pool(name="pKS", bufs=4) as pKS, \
         tc.tile_pool(name="pKG", bufs=2, space="PSUM") as pKG, \
         tc.tile_pool(name="p1p", bufs=2, space="PSUM") as p1p:
        for bi in range(nb1):
            t0 = bi * B1
            nt = min(B1, T1 - t0)
            cvb = p1b.tile([128, B1 * 128], BF16, tag="cvb")
            nc.sync.dma_start(cvb[:, :nt * 128],
                              t["curv1t"][:, t0 * 128:(t0 + nt) * 128])
            qgb = p1b.tile([128, B1 * 128], BF16, tag="qgb")
            kgb = p1b.tile([128, B1 * 128], BF16, tag="kgb")
            for j in range(nt):
                ind_dma(qgb[:, j * 128:(j + 1) * 128], q_own_d[:],
                        qi_s[:, t0 + j:t0 + j + 1])
            otl = p1b.tile([128, B1 * 128], BF16, tag="otl")
            nc.sync.dma_start(otl[:, :nt * 128],
                              t["ohT_lo"][:, t0 * 128:(t0 + nt) * 128])
            oth = p1b.tile([128, B1 * 128], BF16, tag="oth")
            nc.sync.dma_start(oth[:, :nt * 128],
                              t["ohT_hi"][:, t0 * 128:(t0 + nt) * 128])
            for j in range(nt):
                ti = t0 + j
                if ti % T1W == 0:
                    ww = ti // T1W
                    sl = pKS.tile([128, 128], BF16, tag="slab_lo", name="slab_lo")
                    nc.sync.dma_start(
                        sl[:], kv_full[ww * 256:ww * 256 + 128, 0:128])
                    sh = pKS.tile([128, 128], BF16, tag="slab_hi", name="slab_hi")
                    nc.sync.dma_start(
                        sh[:], kv_full[ww * 256 + 128:(ww + 1) * 256, 0:128])
                    _kslab_cur[0] = sl
                    _kslab_cur[1] = sh
                kg = pKG.tile([128, 128], F32, tag="kg")
                nc.tensor.matmul(out=kg[:], lhsT=otl[:, j * 128:(j + 1) * 128],
                                 rhs=_kslab_cur[0][:], start=True, stop=False)
                nc.tensor.matmul(out=kg[:], lhsT=oth[:, j * 128:(j + 1) * 128],
                                 rhs=_kslab_cur[1][:], start=False, stop=True)
                nc.vector.tensor_copy(out=kgb[:, j * 128:(j + 1) * 128],
                                      in_=kg[:])
            # curv @ Wc (per-tile lhsT) into one PSUM block
            psc = p1p.tile([128, B1 * H], F32, tag="psc")
            for j in range(nt):
                nc.tensor.matmul(out=psc[:, j * H:(j + 1) * H],
                                 lhsT=cvb[:, j * 128:(j + 1) * 128],
                                 rhs=wc_s[:], start=True, stop=True)
            # scores for the whole block
            prod = p1.tile([128, B1 * 128], BF16, tag="prod")
            nc.vector.tensor_tensor(out=prod[:, :nt * 128],
                                    in0=qgb[:, :nt * 128],
                                    in1=kgb[:, :nt * 128],
                                    op=mybir.AluOpType.mult)
            qk = p1.tile([128, B1 * H], F32, tag="qk")
            nc.vector.tensor_reduce(
                out=qk[:, :nt * H],
                in_=prod[:, :nt * 128].rearrange("p (q x) -> p q x", x=HD),
                axis=mybir.AxisListType.X, op=mybir.AluOpType.add)
            qks = p1.tile([128, B1 * H], F32, tag="qks")
            nc.vector.scalar_tensor_tensor(out=qks[:, :nt * H],
                                           in0=qk[:, :nt * H],
                                           scalar=0.25, in1=psc[:, :nt * H],
                                           op0=mybir.AluOpType.mult,
                                           op1=mybir.AluOpType.add)
            nc.vector.tensor_tensor(
                out=qks[:, :nt * H].rearrange("p (q h) -> p q h", h=H),
                in0=qks[:, :nt * H].rearrange("p (q h) -> p q h", h=H),
                in1=bc_s[:].rearrange("p (o h) -> p o h", o=1)
                .broadcast_to([128, nt, H]),
                op=mybir.AluOpType.add)
            nc.scalar.activation(out=ex_sb[:, t0 * H:(t0 + nt) * H],
                                 in_=qks[:, :nt * H],
                                 func=mybir.ActivationFunctionType.Exp)
            # one-hot columns for this block
            ohb = p1.tile([128, B1 * 256], BF16, tag="ohb")
            nc.vector.tensor_tensor(
                out=ohb[:, :nt * 256].rearrange("p (q n) -> p q n", n=256),
                in0=trel_s[:, t0:t0 + nt].rearrange("p (q o) -> p q o", o=1)
                .broadcast_to([128, nt, 256]),
                in1=iota256_s[:].rearrange("p (o n) -> p o n", o=1)
                .broadcast_to([128, nt, 256]),
                op=mybir.AluOpType.is_equal)
            for j in range(nt):
                ti = t0 + j
                ex_t = ex_sb[:, ti * H:(ti + 1) * H]
                ww = ti // T1W
                tt1 = ti % T1W
                if tt1 == 0:
                    _psd_cur[0] = p1p.tile([128, H], F32, tag="psd_lo", name="psd_lo")
                    _psd_cur[1] = p1p.tile([128, H], F32, tag="psd_hi", name="psd_hi")
                psd_lo, psd_hi = _psd_cur[0], _psd_cur[1]
                nc.tensor.matmul(out=psd_lo[:],
                                 lhsT=ohb[:, j * 256:j * 256 + 128], rhs=ex_t,
                                 start=(tt1 == 0), stop=(tt1 == T1W - 1))
                nc.tensor.matmul(out=psd_hi[:],
                                 lhsT=ohb[:, j * 256 + 128:(j + 1) * 256],
                                 rhs=ex_t, start=(tt1 == 0),
                                 stop=(tt1 == T1W - 1))
                if tt1 == T1W - 1:
                    nc.vector.tensor_copy(
                        out=den_tab[:, ww * 2 * H:ww * 2 * H + H],
                        in_=psd_lo[:])
                    nc.vector.tensor_copy(
                        out=den_tab[:, ww * 2 * H + H:(ww + 1) * 2 * H],
                        in_=psd_hi[:])
        nc.sync.dma_start(
            den_d[:].rearrange("(w p) h -> p w h", p=128),
            den_tab[:, :NW * H].rearrange("p (w h) -> p w h", h=H))

    # AllReduce denom -> full table on every core
    if not _abl_no_coll:
        nc.gpsimd.collective_compute(
            "AllReduce", mybir.AluOpType.add, replica_groups=rg,
            ins=[den_d.opt()], outs=[den_full_d.opt()])

    # ---------------- Pass 1.5: messages in pass-1 order ----------------
    # msg[slot] = ex[slot] * v[tgt_slot] / den[tgt_slot]; v and 1/den
    # gathered by the same streamed one-hots as the k-gather.
    with tc.tile_pool(name="pRC", bufs=1) as pRC, \
         tc.tile_pool(name="p15", bufs=2) as p15, \
         tc.tile_pool(name="p15b", bufs=2) as p15b, \
         tc.tile_pool(name="pVS", bufs=4) as pVS, \
         tc.tile_pool(name="pVG", bufs=2, space="PSUM") as pVG:
        den_res = pRC.tile([128, NW * H], F32, name="den_res")
        nc.sync.dma_start(den_res[:].rearrange("p (w h) -> p w h", h=H),
                          den_full_d[:].rearrange("(w p) h -> p w h", p=128))
        nc.vector.tensor_scalar_max(den_res[:], den_res[:], 1e-30)
        rec_res = pRC.tile([128, NW * H], F32, name="rec_res")
        nc.vector.reciprocal(out=rec_res[:], in_=den_res[:])
        rec_b = pRC.tile([128, NW * H], BF16, name="rec_b")
        nc.vector.tensor_copy(out=rec_b[:], in_=rec_res[:])
        _vslab_cur = [None, None]
        for bi in range(0 if _abl_no_p15 else nb1):
            t0 = bi * B1
            nt = min(B1, T1 - t0)
            otl2 = p15b.tile([128, B1 * 128], BF16, tag="otl2")
            nc.sync.dma_start(otl2[:, :nt * 128],
                              t["ohT_lo"][:, t0 * 128:(t0 + nt) * 128])
            oth2 = p15b.tile([128, B1 * 128], BF16, tag="oth2")
            nc.sync.dma_start(oth2[:, :nt * 128],
                              t["ohT_hi"][:, t0 * 128:(t0 + nt) * 128])
            msgb = p15.tile([128, B1 * 128], BF16, tag="msgb15")
            prb = p15.tile([128, B1 * H], BF16, tag="prb")
            for j in range(nt):
                ti = t0 + j
                if ti % T1W == 0:
                    ww = ti // T1W
                    # fused [v-slab | 1/den-cols] rhs (136 cols)
                    vsl = pVS.tile([128, 136], BF16, tag="vslab_lo",
                                   name="vslab_lo")
                    nc.sync.dma_start(
                        vsl[:, 0:128],
                        kv_full[ww * 256:ww * 256 + 128, 128:256])
                    nc.vector.tensor_copy(
                        out=vsl[:, 128:136],
                        in_=rec_b[:, (2 * ww) * H:(2 * ww + 1) * H])
                    vsh = pVS.tile([128, 136], BF16, tag="vslab_hi",
                                   name="vslab_hi")
                    nc.sync.dma_start(
                        vsh[:, 0:128],
                        kv_full[ww * 256 + 128:(ww + 1) * 256, 128:256])
                    nc.vector.tensor_copy(
                        out=vsh[:, 128:136],
                        in_=rec_b[:, (2 * ww + 1) * H:(2 * ww + 2) * H])
                    _vslab_cur[0] = vsl
                    _vslab_cur[1] = vsh
                vgr = pVG.tile([128, 136], F32, tag="vgr")
                nc.tensor.matmul(out=vgr[:],
                                 lhsT=otl2[:, j * 128:(j + 1) * 128],
                                 rhs=_vslab_cur[0][:], start=True, stop=False)
                nc.tensor.matmul(out=vgr[:],
                                 lhsT=oth2[:, j * 128:(j + 1) * 128],
                                 rhs=_vslab_cur[1][:], start=False, stop=True)
                ti8 = ti * H
                nc.vector.tensor_tensor(out=prb[:, j * H:(j + 1) * H],
                                        in0=ex_sb[:, ti8:ti8 + H],
                                        in1=vgr[:, 128:136],
                                        op=mybir.AluOpType.mult)
                nc.vector.tensor_tensor(
                    out=msgb[:, j * 128:(j + 1) * 128]
                    .rearrange("p (h x) -> p h x", h=H),
                    in0=vgr[:, 0:128].rearrange("p (h x) -> p h x", h=H),
                    in1=prb[:, j * H:(j + 1) * H].broadcast_to([128, H, HD]),
                    op=mybir.AluOpType.mult)
            nc.sync.dma_start(
                msg_d[:].rearrange("(p t2) e -> p (t2 e)", p=128)
                [:, t0 * 128:(t0 + nt) * 128],
                msgb[:, :nt * 128])

    # ---------------- Pass 2 + Phase D ----------------
    msg_flat2 = msg_d[:]
    B2 = 16
    with tc.tile_pool(name="p2", bufs=2) as p2, \
         tc.tile_pool(name="p2b", bufs=2) as p2b, \
         tc.tile_pool(name="p2p", bufs=2, space="PSUM") as p2p, \
         tc.tile_pool(name="pD", bufs=2) as pD, \
         tc.tile_pool(name="pDp", bufs=1, space="PSUM") as pDp:
        nb2 = (T2 + B2 - 1) // B2
        # prefetch loop is flat over tiles; window boundaries align since
        # T2W*W tiles total and windows are contiguous runs of T2W tiles.
        for bi in range(nb2):
            t0 = bi * B2
            nt = min(B2, T2 - t0)
            # padded slots point at pad pass-1 rows whose msg is exactly 0,
            # so no mask multiply is needed.
            mgb = p2b.tile([128, B2 * 128], BF16, tag="mgb")
            for j in range(0, nt, 2):
                ind_dma(mgb[:, j * 128:(j + 2) * 128], msg_flat2,
                        expos_s[:, (t0 + j) // 2:(t0 + j) // 2 + 1])
            # one-hot src columns for the block
            oh2b = p2.tile([128, B2 * 128], BF16, tag="oh2b")
            nc.vector.tensor_tensor(
                out=oh2b[:, :nt * 128].rearrange("p (q n) -> p q n", n=128),
                in0=srcl2_s[:, t0:t0 + nt].rearrange("p (q o) -> p q o", o=1)
                .broadcast_to([128, nt, 128]),
                in1=iota128_s[:].rearrange("p (o n) -> p o n", o=1)
                .broadcast_to([128, nt, 128]),
                op=mybir.AluOpType.is_equal)
            for j in range(nt):
                ti = t0 + j
                w = ti // T2W
                tt = ti % T2W
                if tt == 0:
                    aggT = p2p.tile([128, 128], F32, tag="aggT")
                    tc._aggT_cur = aggT  # stash
                aggT = tc._aggT_cur
                nc.tensor.matmul(out=aggT[:],
                                 lhsT=mgb[:, j * 128:(j + 1) * 128],
                                 rhs=oh2b[:, j * 128:(j + 1) * 128],
                                 start=(tt == 0), stop=(tt == T2W - 1))
                if tt == T2W - 1:
                    # -------- Phase D for window w --------
                    aggT_sb = pD.tile([128, 128], BF16, tag="aggT_sb")
                    nc.vector.tensor_copy(out=aggT_sb[:], in_=aggT[:])
                    attn = pDp.tile([128, 128], F32, tag="attn")
                    nc.tensor.matmul(out=attn[:], lhsT=aggT_sb[:], rhs=wo_s[:],
                                     start=True, stop=False)
                    nc.tensor.matmul(out=attn[:], lhsT=ones_s[:], rhs=bo_s[:],
                                     start=False, stop=True)
                    xw2 = pD.tile([128, 128], F32, tag="xw2")
                    nc.sync.dma_start(xw2[:],
                                      t["x_own"][w * 128:(w + 1) * 128, :])
                    x1 = pD.tile([128, 128], F32, tag="x1")
                    nc.vector.tensor_tensor(out=x1[:], in0=xw2[:], in1=attn[:],
                                            op=mybir.AluOpType.add)
                    # LN2 stats
                    s1b = pD.tile([128, 1], F32, tag="s1b")
                    nc.vector.tensor_reduce(out=s1b[:], in_=x1[:],
                                            axis=mybir.AxisListType.X,
                                            op=mybir.AluOpType.add)
                    sqb = pD.tile([128, 128], F32, tag="sqb")
                    nc.scalar.activation(
                        out=sqb[:], in_=x1[:],
                        func=mybir.ActivationFunctionType.Square)
                    s2b = pD.tile([128, 1], F32, tag="s2b")
                    nc.vector.tensor_reduce(out=s2b[:], in_=sqb[:],
                                            axis=mybir.AxisListType.X,
                                            op=mybir.AluOpType.add)
                    mb = pD.tile([128, 1], F32, tag="mb")
                    nc.vector.tensor_scalar_mul(mb[:], s1b[:], 1.0 / 128.0)
                    m2b = pD.tile([128, 1], F32, tag="m2b")
                    nc.vector.tensor_tensor(out=m2b[:], in0=mb[:], in1=mb[:],
                                            op=mybir.AluOpType.mult)
                    varb = pD.tile([128, 1], F32, tag="varb")
                    nc.vector.scalar_tensor_tensor(
                        out=varb[:], in0=s2b[:], scalar=1.0 / 128.0, in1=m2b[:],
                        op0=mybir.AluOpType.mult, op1=mybir.AluOpType.subtract)
                    stdb = pD.tile([128, 1], F32, tag="stdb")
                    nc.scalar.activation(
                        out=stdb[:], in_=varb[:],
                        func=mybir.ActivationFunctionType.Sqrt,
                        bias=eps_col[:])
                    rstdb = pD.tile([128, 1], F32, tag="rstdb")
                    nc.vector.reciprocal(out=rstdb[:], in_=stdb[:])
                    negmb = pD.tile([128, 1], F32, tag="negmb")
                    nc.vector.tensor_scalar_mul(negmb[:], mb[:], -1.0)
                    nm_psb = pDp.tile([128, 128], F32, tag="tr_psb")
                    nc.tensor.transpose(out=nm_psb[:1, :], in_=negmb[:],
                                        identity=ident[:])
                    st_psb = pDp.tile([128, 128], F32, tag="tr_psb")
                    nc.tensor.transpose(out=st_psb[:1, :], in_=stdb[:],
                                        identity=ident[:])
                    negm_rb = pD.tile([1, 128], BF16, tag="negm_rb")
                    nc.vector.tensor_copy(out=negm_rb[:], in_=nm_psb[:1, :])
                    std_rb = pD.tile([1, 128], BF16, tag="std_rb")
                    nc.vector.tensor_copy(out=std_rb[:], in_=st_psb[:1, :])
                    # x1T (bf16 for the FFN matmuls)
                    x1T_ps = pDp.tile([128, 128], F32, tag="tr_psb")
                    nc.tensor.transpose(out=x1T_ps[:], in_=x1[:],
                                        identity=ident[:])
                    x1T = pD.tile([128, 128], BF16, tag="x1T")
                    nc.vector.tensor_copy(out=x1T[:], in_=x1T_ps[:])
                    hp = pDp.tile([128, 512], F32, tag="hp")
                    nc.tensor.matmul(out=hp[:], lhsT=x1T[:], rhs=w1_s[:],
                                     start=True, stop=False)
                    nc.tensor.matmul(out=hp[:], lhsT=negm_rb[:], rhs=r11_s[:],
                                     start=False, stop=False)
                    nc.tensor.matmul(out=hp[:], lhsT=std_rb[:], rhs=b12_s[:],
                                     start=False, stop=True)
                    hsb = pD.tile([128, 512], BF16, tag="hsb")
                    nc.scalar.activation(out=hsb[:], in_=hp[:],
                                         func=mybir.ActivationFunctionType.Relu,
                                         scale=rstdb[:])
                    ffn = pDp.tile([128, 128], F32, tag="ffn")
                    for cch in range(4):
                        hT_ps = pDp.tile([128, 128], BF16, tag="tr_psb2")
                        nc.tensor.transpose(
                            out=hT_ps[:], in_=hsb[:, cch * 128:(cch + 1) * 128],
                            identity=ident_b[:])
                        hT = pD.tile([128, 128], BF16, tag="hT")
                        nc.vector.tensor_copy(out=hT[:], in_=hT_ps[:])
                        nc.tensor.matmul(out=ffn[:], lhsT=hT[:],
                                         rhs=w2_s[:, cch * 128:(cch + 1) * 128],
                                         start=(cch == 0), stop=False)
                    nc.tensor.matmul(out=ffn[:], lhsT=ones_s[:], rhs=b2_s[:],
                                     start=False, stop=True)
                    outw = pD.tile([128, 128], F32, tag="outw")
                    nc.vector.tensor_tensor(out=outw[:], in0=x1[:], in1=ffn[:],
                                            op=mybir.AluOpType.add)
                    nc.sync.dma_start(t["out"][w * 128:(w + 1) * 128, :],
                                      outw[:])

    ctx.close()


def build_program(pp, nc_factory):
    """Create Bacc, declare IO, build tile program, compile. Returns nc."""
    import concourse.tile as tile
    nc = nc_factory()
    t = declare_io(nc, pp)
    with tile.TileContext(nc) as tc:
        build(tc, t, pp)
    nc.compile()
    return nc


# --------------------------------------------------------------------------
# Harness entry point
# --------------------------------------------------------------------------

NCORES = 8
W_PER_CORE = 49  # 8*49*128 = 50176 >= 50000 nodes


def _run_spmd_timed(nc, in_maps, n_cores, reps=10, chain=8):
    """Execute the SPMD program via PJRT with device-staged inputs; returns
    (per-core results, steady-state per-execution time in ns).

    The axon tunnel adds ~75 ms of fixed dispatch round-trip latency per
    synchronous call, unrelated to on-device execution. We measure T(1) and
    T(1+chain) where the extra executions are chained back-to-back on device
    (each feeding its output buffer to the next call), and report
    (T(1+chain) - T(1)) / chain: the marginal hardware execution time.
    """
    import time

    import jax
    from jax.experimental.shard_map import shard_map
    from jax.sharding import Mesh, NamedSharding, PartitionSpec

    from concourse.bass2jax import (_bass_exec_p, install_neuronx_cc_hook,
                                    partition_id_tensor)

    install_neuronx_cc_hook()
    partition_name = (nc.partition_id_tensor.name
                      if nc.partition_id_tensor else None)
    in_names, out_names, out_avals, zero_outs = [], [], [], []
    for alloc in nc.m.functions[0].allocations:
        if not isinstance(alloc, mybir.MemoryLocationSet):
            continue
        name = alloc.memorylocations[0].name
        if alloc.kind == "ExternalInput":
            if name != partition_name:
                in_names.append(name)
        elif alloc.kind == "ExternalOutput":
            shape = tuple(alloc.tensor_shape)
            dtype = mybir.dt.np(alloc.dtype)
            out_names.append(name)
            out_avals.append(jax.core.ShapedArray(shape, dtype))
            zero_outs.append(np.zeros(shape, dtype))
    n_params = len(in_names)
    n_outs = len(out_avals)
    in_names.extend(out_names)
    if partition_name is not None:
        in_names.append(partition_name)
    donate = tuple(range(n_params, n_params + n_outs))

    def _body(*args):
        operands = list(args)
        if partition_name is not None:
            operands.append(partition_id_tensor())
        outs = _bass_exec_p.bind(
            *operands, out_avals=tuple(out_avals), in_names=tuple(in_names),
            out_names=tuple(out_names), lowering_input_output_aliases=(),
            sim_require_finite=True, sim_require_nnan=True, nc=nc)
        return tuple(outs)

    devices = jax.devices()[:n_cores]
    mesh = Mesh(np.asarray(devices), ("core",))
    sharding = NamedSharding(mesh, PartitionSpec("core"))
    in_specs = (PartitionSpec("core"),) * (n_params + n_outs)
    out_specs = (PartitionSpec("core"),) * len(out_names)
    sharded = jax.jit(
        shard_map(_body, mesh=mesh, in_specs=in_specs, out_specs=out_specs,
                  check_rep=False),
        donate_argnums=donate, keep_unused=True)
    concat_in = [
        np.concatenate([np.asarray(in_maps[c][in_names[i]])
                        for c in range(n_cores)], axis=0)
        for i in range(n_params)]
    dev_in = [jax.device_put(a, sharding) for a in concat_in]

    def fresh_zeros():
        zs = [jax.device_put(
            np.zeros((n_cores * z.shape[0], *z.shape[1:]), z.dtype), sharding)
            for z in zero_outs]
        jax.block_until_ready(zs)
        return zs

    out_arrs = sharded(*dev_in, *fresh_zeros())
    jax.block_until_ready(out_arrs)
    results = [
        {name: np.asarray(out_arrs[i]).reshape(n_cores, *out_avals[i].shape)[c]
         for i, name in enumerate(out_names)}
        for c in range(n_cores)]

    def run_chain(n_execs):
        o = tuple(fresh_zeros())
        t0 = time.perf_counter()
        for _ in range(n_execs):
            o = sharded(*dev_in, *o)
        jax.block_until_ready(o)
        return time.perf_counter() - t0

    best = None
    for _ in range(max(reps, 0)):
        t_one = run_chain(1)
        t_many = run_chain(1 + chain)
        marginal = (t_many - t_one) / chain
        best = marginal if best is None or marginal < best else best
    return results, (None if best is None else int(best * 1e9))


def kernel(**inputs):
    import sys
    if "/opt/trn_rl_repo" not in sys.path:
        sys.path.insert(0, "/opt/trn_rl_repo")
    import concourse.bacc as bacc

    x = np.asarray(inputs["x"], np.float32)
    edge_index = np.asarray(inputs["edge_index"])
    curv = np.asarray(inputs["curvature_embeddings"], np.float32)
    weights = {k: np.asarray(v) for k, v in inputs.items()
               if k not in ("x", "edge_index", "curvature_embeddings")}

    pp, in_maps = host_prep(x, edge_index, curv, weights, NCORES, W_PER_CORE)
    nc = build_program(pp, lambda: bacc.Bacc(
        "TRN2", target_bir_lowering=False, debug=False, num_devices=NCORES,
        num_swdge_queues=4))
    results, best_ns = _run_spmd_timed(nc, in_maps, NCORES)
    kernel.last_exec_ns = best_ns
    out = np.concatenate([results[c]["out"] for c in range(NCORES)],
                         axis=0)[:x.shape[0]]
    return np.ascontiguousarray(out, dtype=np.float32)


# revision 24
# speedup vs baseline: 1.0070x; 1.0008x over previous
"""Curvphormer GNN layer as a Bass/Tile SPMD kernel for TRN2.

Design (per core c of NCORES, equal node ranges of W windows x 128 nodes;
edges sharded by src-window range):
 - Phase A: fused-LN q/k/v build for own node range (LN folded into matmuls);
   q/k/v tables in bf16. AllGather(k), AllGather(v).
 - Pass 1 (edges grouped by 256-node tgt windows, 16-tile blocks):
   q[src] per-tile indirect-DMA gather (own table); k[tgt] via one-hot
   MATMULS (host-streamed transposed one-hots x streamed k-window slabs) --
   no per-edge DMA; scores = q.k/4 + curv@Wc + bc (block-fused DVE ops),
   ex = exp(score) (max-free softmax: scores are O(1) by construction);
   segment-sum of ex by tgt via one-hot matmuls into per-wide-window PSUM ->
   SBUF denominator table. Padding handled by an out-of-range one-hot index.
 - AllReduce(denominators) -> full [N,H] table on every core.
 - Pass 1.5 (same tgt-grouped order): v[tgt] and 1/den[tgt] gathered by the
   same one-hot matmuls; msg[slot] = ex * v/den written contiguously to DRAM
   in pass-1 slot order. No per-edge DMA.
 - Pass 2 (edges grouped by own src-window, fixed T2W tiles per window):
   single per-tile indirect-DMA gather of msg rows (by pass-1 position);
   mask + aggregate transposed agg via one-hot bf16 matmuls in PSUM/window.
 - Phase D (fused per window): out = x1 + FFN(LN2(x1)), x1 = x + agg@Wo + bo;
   FFN/attn-out matmuls in bf16.

Indirect DMA note: the hardware honors only single-column [128,1] offset APs
(one offset per partition per call); multi-column offset batching silently
degrades to consecutive-row reads. All remaining indirect gathers therefore
use per-tile single-column offsets; everything else was restructured into
one-hot matmuls / contiguous streams.

Timing: chained-dispatch marginal. A single dispatch over the axon tunnel has
~75 ms of fixed client<->device round-trip latency that is unrelated to kernel
execution; we measure T(1) and T(1+B) where the B extra executions are chained
back-to-back on device (each feeding its donated output buffer to the next
call), and report (T(1+B)-T(1))/B -- the steady-state hardware execution time
per run.
"""

import sys
if "/opt/trn_rl_repo" not in sys.path:
    sys.path.insert(0, "/opt/trn_rl_repo")

import numpy as np

import concourse.bass as bass
import concourse.mybir as mybir
from concourse.masks import make_identity

F32 = mybir.dt.float32
BF16 = mybir.dt.bfloat16
I32 = mybir.dt.int32

D = 128
H = 8
HD = 16
LN_EPS = 1e-5
NOMATCH = 300.0  # one-hot index for padded slots: never matches iota < 256


class P:
    """Static program parameters (identical across cores -> SPMD safe)."""

    def __init__(self, ncores, W, T1, T2W):
        self.ncores = ncores
        self.W = W              # windows (of 128 nodes) per core
        self.T1 = T1            # pass-1 tiles (128 edges each) per core
        self.T2W = T2W          # pass-2 tiles per window
        self.nodes_pc = W * 128
        self.npad = ncores * W * 128
        self.T2 = W * T2W


def _bf16(a):
    import ml_dtypes
    return np.asarray(a, dtype=ml_dtypes.bfloat16)


# --------------------------------------------------------------------------
# Host-side preprocessing
# --------------------------------------------------------------------------

def host_prep(x, edge_index, curv, weights, ncores, W):
    """Build per-core input maps. weights: dict with raw reference weights."""
    N = x.shape[0]
    E = edge_index.shape[1]
    nodes_pc = W * 128
    npad = ncores * nodes_pc
    assert npad >= N

    src = np.asarray(edge_index[0], dtype=np.int64)
    tgt = np.asarray(edge_index[1], dtype=np.int64)
    x_pad = np.zeros((npad, D), dtype=np.float32)
    x_pad[:N] = x

    core_of = (src // 128) // W
    order_by_core = np.argsort(core_of, kind="stable")
    counts = np.bincount(core_of, minlength=ncores)
    splits = np.split(order_by_core, np.cumsum(counts)[:-1])

    # pass-1: edges grouped by 256-node wide tgt-windows, padded to a fixed
    # tile count per wide-window (static, SPMD-uniform).
    NWW = (ncores * W + 1) // 2  # wide windows of 256 nodes
    T1W = 0
    for c in range(ncores):
        cnt = np.bincount(tgt[splits[c]] // 256, minlength=NWW)
        T1W = max(T1W, int(np.ceil(cnt.max() / 128)))
    T1 = NWW * T1W
    # pass-2: max tiles per (core, window)
    T2W = 0
    for c in range(ncores):
        e_c = splits[c]
        w_loc = (src[e_c] // 128) - c * W
        cnt = np.bincount(w_loc, minlength=W)
        T2W = max(T2W, int(np.ceil(cnt.max() / 128)))
    # paired pass-2: per-window entries (pairs + singles) observed at
    # ~0.54x sub-tile count; 0.55x + ceil gives ~16% headroom.
    T2W = 2 * int(np.ceil(T2W * 0.55))
    T2 = W * T2W

    pp = P(ncores, W, T1, T2W)
    pp.NWW = NWW
    pp.T1W = T1W

    # LN-folded weights (host)
    g1, be1, g2, be2 = weights["g1"], weights["be1"], weights["g2"], weights["be2"]

    def fold(Wm, b):
        Wp = (g1[:, None] * Wm).astype(np.float32)
        r1 = Wp.sum(axis=0).astype(np.float32)
        b2 = (be1 @ Wm + b).astype(np.float32)
        return Wp, r1, b2

    wq, r1q, bq2 = fold(weights["Wq"], weights["bq"])
    wk, r1k, bk2 = fold(weights["Wk"], weights["bk"])
    wv, r1v, bv2 = fold(weights["Wv"], weights["bv"])
    w1 = (g2[:, None] * weights["W1"]).astype(np.float32)
    r11 = w1.sum(axis=0).astype(np.float32)
    b12 = (be2 @ weights["W1"] + weights["b1"]).astype(np.float32)

    common = {
        "wq": wq, "wk": wk, "wv": wv,
        "wc": _bf16(weights["Wc"]),
        "wo": _bf16(weights["Wo"]),
        "w1": _bf16(w1),
        "w2": _bf16(np.ascontiguousarray(
            weights["W2"].astype(np.float32).reshape(4, 128, D)
            .transpose(1, 0, 2).reshape(128, 4 * D))),
        "r1q": r1q[None, :], "r1k": r1k[None, :], "r1v": r1v[None, :],
        "bq2": bq2[None, :], "bk2": bk2[None, :], "bv2": bv2[None, :],
        "bc_b": np.tile(weights["bc"].astype(np.float32)[None, :], (128, 1)),
        "bo_r": _bf16(weights["bo"])[None, :],
        "r11": _bf16(r11)[None, :], "b12": _bf16(b12)[None, :],
        "b2_r": _bf16(weights["b2"])[None, :],
        "ones_r": _bf16(np.ones((1, D), np.float32)),
        "iota256": np.tile(np.arange(256, dtype=np.float32)[None, :], (128, 1)),
        "iota128": np.tile(np.arange(128, dtype=np.float32)[None, :], (128, 1)),
    }

    in_maps = []
    for c in range(ncores):
        e_c = splits[c]
        L = len(e_c)
        # ---- pass 1: group by wide tgt-window, fixed T1W tiles each ----
        NWW, T1W = pp.NWW, pp.T1W
        S1 = T1 * 128
        tgt1 = np.zeros(S1, np.int64)
        src1 = np.zeros(S1, np.int64)
        real1 = np.zeros(S1, bool)
        slot1_of_edge = np.zeros(E, np.int64)
        ww_of = tgt[e_c] // 256
        w_of_all = (src // 128) - c * W
        # per-window entry lists for paired pass-2 gathers:
        # entry = (rowpair, liveA(edge or -1), liveB(edge or -1))
        entries_w = [[] for _ in range(W)]
        for ww in range(NWW):
            ew = e_c[ww_of == ww]
            base_t = ww * T1W
            # domino (pair) cells start at even GLOBAL tile parity
            c0 = 0 if (base_t % 2 == 0) else 1
            dom_cols = [cc for cc in (c0, c0 + 2) if cc + 1 < T1W]
            dominoes = [(p, cc) for p in range(128) for cc in dom_cols]
            used = np.zeros((T1W, 128), bool)
            ndom = 0
            placed = []  # (edge, lt, p)
            wv = w_of_all[ew]
            order = np.argsort(wv, kind="stable")
            ew_s = ew[order]; wv_s = wv[order]
            i = 0
            singles = []
            while i < len(ew_s):
                jx = i
                while jx < len(ew_s) and wv_s[jx] == wv_s[i]:
                    jx += 1
                grp = ew_s[i:jx]
                wloc = int(wv_s[i])
                gi = 0
                while gi + 1 < len(grp) and ndom < len(dominoes):
                    p, cc = dominoes[ndom]; ndom += 1
                    eA, eB = int(grp[gi]), int(grp[gi + 1]); gi += 2
                    placed.append((eA, cc, p)); placed.append((eB, cc + 1, p))
                    used[cc, p] = True; used[cc + 1, p] = True
                    rp = (p * T1 + base_t + cc) // 2
                    entries_w[wloc].append((rp, eA, eB))
                for e1 in grp[gi:]:
                    singles.append((int(e1), wloc))
                i = jx
            free_cells = [(lt, p) for lt in range(T1W) for p in range(128)
                          if not used[lt, p]]
            assert len(free_cells) >= len(singles)
            for (e1, wloc), (lt, p) in zip(singles, free_cells):
                placed.append((e1, lt, p))
                tg = base_t + lt
                rp = (p * T1 + tg) // 2
                if tg % 2 == 0:
                    entries_w[wloc].append((rp, e1, -1))
                else:
                    entries_w[wloc].append((rp, -1, e1))
            for e1, lt, p in placed:
                s_ = (base_t + lt) * 128 + p
                tgt1[s_] = tgt[e1]
                src1[s_] = src[e1]
                real1[s_] = True
                slot1_of_edge[e1] = s_

        wwin1 = np.repeat(np.arange(T1) // T1W, 128)  # wide window per slot
        tgt_rel = np.where(real1, tgt1 - wwin1 * 256, NOMATCH)
        assert tgt_rel.min() >= 0 and tgt_rel.max() <= NOMATCH

        curv1 = np.zeros((S1, D), np.float32)
        if L:
            curv1[slot1_of_edge[e_c]] = curv[e_c]
        # [128 d, T1*128] partition-major layout: row d, col t*128+e
        curv1t = _bf16(np.ascontiguousarray(
            curv1.reshape(T1, 128, D).transpose(2, 0, 1).reshape(D, T1 * 128)))

        def lay(a, T):  # [T*128] -> [128, T]
            return np.ascontiguousarray(a.reshape(T, 128).T)

        qi = lay(np.where(real1, src1 - c * nodes_pc, 0).astype(np.int32), T1)
        trel = lay(tgt_rel.astype(np.float32), T1)
        # transposed one-hots for the k-gather matmuls: ohT[n, slot] =
        # (tgt_rel[slot] == n) / (== n+128); slot s maps to column s
        # (device tile t = s//128, partition e = s%128 -> col t*128+e = s).
        import ml_dtypes as _mld
        ohT_lo = np.zeros((128, S1), _mld.bfloat16)
        ohT_hi = np.zeros((128, S1), _mld.bfloat16)
        s_idx = np.arange(S1)
        reli = tgt_rel.astype(np.int64)
        m_lo = real1 & (reli < 128)
        m_hi = real1 & (reli >= 128) & (reli < 256)
        ohT_lo[reli[m_lo], s_idx[m_lo]] = 1
        ohT_hi[reli[m_hi] - 128, s_idx[m_hi]] = 1

        # ---- pass 2: paired entries per src-window ----
        # pair-tile m of window w covers sub-tiles (2m, 2m+1); one 512B-row
        # descriptor per partition fetches both msgs; dead sub-slots get
        # srcl2=NOMATCH so their junk msg scatters into a zero one-hot.
        T2P = T2 // 2
        T2WP = T2W // 2
        S2 = T2 * 128
        exp_pos = np.zeros(T2P * 128, np.int64)
        sl2 = np.full(S2, NOMATCH, np.float32)
        for w in range(W):
            ents = entries_w[w]
            assert len(ents) <= T2WP * 128, (len(ents), T2WP * 128)
            for i, (rp, eA, eB) in enumerate(ents):
                m_ = w * T2WP + i // 128
                p2 = i % 128
                exp_pos[m_ * 128 + p2] = rp
                for half, e1 in ((0, eA), (1, eB)):
                    if e1 >= 0:
                        sl2[(2 * m_ + half) * 128 + p2] = \
                            src[e1] - (c * W + w) * 128
        expos = lay(exp_pos.astype(np.int32), T2P)
        srcl2 = lay(sl2, T2)

        x_own = np.ascontiguousarray(x_pad[c * nodes_pc:(c + 1) * nodes_pc])
        xT_own = np.ascontiguousarray(x_own.T)

        m = dict(common)
        m.update({
            "x_own": x_own, "xT_own": xT_own,
            "curv1t": curv1t,
            "qi": qi, "trel": trel,
            "ohT_lo": ohT_lo, "ohT_hi": ohT_hi,
            "expos": expos, "srcl2": srcl2,
        })
        in_maps.append(m)

    return pp, in_maps


# --------------------------------------------------------------------------
# Device program
# --------------------------------------------------------------------------

def declare_io(nc, pp):
    """Declare all ExternalInput/Output dram tensors; returns dict of APs."""
    t = {}

    def din(name, shape, dt=F32):
        t[name] = nc.dram_tensor(name, list(shape), dt, kind="ExternalInput").ap()

    W, T1, T2 = pp.W, pp.T1, pp.T2
    din("x_own", (pp.nodes_pc, D)); din("xT_own", (D, pp.nodes_pc))
    din("curv1t", (D, T1 * 128), BF16)
    din("qi", (128, T1), I32)
    din("trel", (128, T1))
    din("ohT_lo", (128, T1 * 128), BF16); din("ohT_hi", (128, T1 * 128), BF16)
    din("expos", (128, T2 // 2), I32)
    din("srcl2", (128, T2))
    for n, shp, dt in [("wq", (D, D), F32), ("wk", (D, D), F32),
                       ("wv", (D, D), F32),
                       ("wc", (D, H), BF16), ("wo", (D, D), BF16),
                       ("w1", (D, 4 * D), BF16), ("w2", (D, 4 * D), BF16),
                       ("r1q", (1, D), F32), ("r1k", (1, D), F32),
                       ("r1v", (1, D), F32),
                       ("bq2", (1, D), F32), ("bk2", (1, D), F32),
                       ("bv2", (1, D), F32),
                       ("bc_b", (128, H), F32), ("bo_r", (1, D), BF16),
                       ("r11", (1, 4 * D), BF16), ("b12", (1, 4 * D), BF16),
                       ("b2_r", (1, D), BF16),
                       ("ones_r", (1, D), BF16),
                       ("iota256", (128, 256), F32),
                       ("iota128", (128, 128), F32)]:
        din(n, shp, dt)
    t["out"] = nc.dram_tensor("out", [pp.nodes_pc, D], F32,
                              kind="ExternalOutput").ap()
    return t


def build(tc, t, pp):
    import os as _os
    _abl_no_coll = bool(_os.environ.get("ABL_NO_COLL"))
    _abl_no_p15 = bool(_os.environ.get("ABL_NO_P15"))
    nc = tc.nc
    _rr = [0]

    def ind_dma(out, in_, off_ap):
        import os
        if os.environ.get("ABL_NO_GATHER"):
            return None
        inst = nc.gpsimd.indirect_dma_start(
            out=out, out_offset=None, in_=in_,
            in_offset=bass.IndirectOffsetOnAxis(ap=off_ap, axis=0))
        q = _rr[0] % 4
        _rr[0] += 1
        if q:
            inst.ins.queue = f"qPoolDynamic{q}"
        return inst
    W, T1, T2W, T2 = pp.W, pp.T1, pp.T2W, pp.T2
    NW = pp.ncores * W  # total windows (392)
    rg = [list(range(pp.ncores))]
    from contextlib import ExitStack
    ctx = ExitStack()

    # internal DRAM
    q_own_d, _ = tc.tile([pp.nodes_pc, D], BF16, space="DRAM", name="q_own_d")
    kv_own_d, _ = tc.tile([pp.nodes_pc, 2 * D], BF16, space="DRAM",
                          name="kv_own_d")
    kv_full, _ = tc.tile([pp.npad, 2 * D], BF16, space="DRAM",
                         addr_space="Shared", name="kv_full")
    den_d, _ = tc.tile([NW * 128, H], F32, space="DRAM", name="den_d")
    den_full_d, _ = tc.tile([NW * 128, H], F32, space="DRAM",
                            addr_space="Shared", name="den_full_d")
    msg_d, _ = tc.tile([128 * (T1 // 2), 256], BF16, space="DRAM",
                       name="msg_d")

    const = ctx.enter_context(tc.tile_pool(name="const", bufs=1))

    def load_const(name, dt=None, src=None):
        ap = t[name] if src is None else src
        shp = list(ap.shape)
        tl = const.tile(shp, dt or ap.dtype, name=f"c_{name}")
        nc.sync.dma_start(tl[:], ap[:])
        return tl

    wq_s = load_const("wq"); wk_s = load_const("wk"); wv_s = load_const("wv")
    wc_s = load_const("wc"); wo_s = load_const("wo"); w1_s = load_const("w1")
    w2_s = load_const("w2")
    r1q_s = load_const("r1q"); r1k_s = load_const("r1k"); r1v_s = load_const("r1v")
    bq2_s = load_const("bq2"); bk2_s = load_const("bk2"); bv2_s = load_const("bv2")
    bc_s = load_const("bc_b"); bo_s = load_const("bo_r")
    r11_s = load_const("r11"); b12_s = load_const("b12"); b2_s = load_const("b2_r")
    ones_s = load_const("ones_r")
    qi_s = load_const("qi")
    expos_s = load_const("expos")
    srcl2_f = load_const("srcl2")
    trel_f = load_const("trel")
    iota256_f = load_const("iota256")
    iota128_f = load_const("iota128")

    ident = const.tile([128, 128], F32, name="ident")
    make_identity(nc, ident[:])
    ident_b = const.tile([128, 128], BF16, name="ident_b")
    nc.vector.tensor_copy(out=ident_b[:], in_=ident[:])
    eps_col = const.tile([128, 1], F32, name="eps_col")
    nc.vector.memset(eps_col[:], LN_EPS)

    # bf16 copies for the one-hot / message paths
    trel_s = const.tile([128, T1], BF16, name="trel_b")
    nc.vector.tensor_copy(out=trel_s[:], in_=trel_f[:])
    iota256_s = const.tile([128, 256], BF16, name="iota256_b")
    nc.vector.tensor_copy(out=iota256_s[:], in_=iota256_f[:])
    iota128_s = const.tile([128, 128], BF16, name="iota128_b")
    nc.vector.tensor_copy(out=iota128_s[:], in_=iota128_f[:])
    srcl2_s = const.tile([128, T2], BF16, name="srcl2_b")
    nc.vector.tensor_copy(out=srcl2_s[:], in_=srcl2_f[:])

    # residents
    v_res = const.tile([128, W * 128], BF16, name="v_res")
    ex_sb = const.tile([128, T1 * H], BF16, name="ex_sb")
    den_tab = const.tile([128, (NW + 1) * H], F32, name="den_tab")
    nc.vector.memset(den_tab[:], 0.0)

    # ---------------- Phase A: q/k/v for own windows ----------------
    with tc.tile_pool(name="pA", bufs=2) as pA, \
         tc.tile_pool(name="pAp", bufs=1, space="PSUM") as pAp:
        for w in range(W):
            xw = pA.tile([128, 128], F32, tag="xw")
            nc.sync.dma_start(xw[:], t["x_own"][w * 128:(w + 1) * 128, :])
            xTw = pA.tile([128, 128], F32, tag="xTw")
            nc.sync.dma_start(xTw[:], t["xT_own"][:, w * 128:(w + 1) * 128])
            # stats
            s1 = pA.tile([128, 1], F32, tag="s1")
            nc.vector.tensor_reduce(out=s1[:], in_=xw[:],
                                    axis=mybir.AxisListType.X,
                                    op=mybir.AluOpType.add)
            sq = pA.tile([128, 128], F32, tag="sq")
            nc.scalar.activation(out=sq[:], in_=xw[:],
                                 func=mybir.ActivationFunctionType.Square)
            s2 = pA.tile([128, 1], F32, tag="s2")
            nc.vector.tensor_reduce(out=s2[:], in_=sq[:],
                                    axis=mybir.AxisListType.X,
                                    op=mybir.AluOpType.add)
            mcol = pA.tile([128, 1], F32, tag="mcol")
            nc.vector.tensor_scalar_mul(mcol[:], s1[:], 1.0 / 128.0)
            m2c = pA.tile([128, 1], F32, tag="m2c")
            nc.vector.tensor_tensor(out=m2c[:], in0=mcol[:], in1=mcol[:],
                                    op=mybir.AluOpType.mult)
            var = pA.tile([128, 1], F32, tag="var")
            nc.vector.scalar_tensor_tensor(out=var[:], in0=s2[:],
                                           scalar=1.0 / 128.0, in1=m2c[:],
                                           op0=mybir.AluOpType.mult,
                                           op1=mybir.AluOpType.subtract)
            stdc = pA.tile([128, 1], F32, tag="stdc")
            nc.scalar.activation(out=stdc[:], in_=var[:],
                                 func=mybir.ActivationFunctionType.Sqrt,
                                 bias=eps_col[:])
            rstd = pA.tile([128, 1], F32, tag="rstd")
            nc.vector.reciprocal(out=rstd[:], in_=stdc[:])
            negm = pA.tile([128, 1], F32, tag="negm")
            nc.vector.tensor_scalar_mul(negm[:], mcol[:], -1.0)
            nm_ps = pAp.tile([128, 128], F32, tag="tr_ps")
            nc.tensor.transpose(out=nm_ps[:1, :], in_=negm[:], identity=ident[:])
            st_ps = pAp.tile([128, 128], F32, tag="tr_ps")
            nc.tensor.transpose(out=st_ps[:1, :], in_=stdc[:], identity=ident[:])
            negm_r = pA.tile([1, 128], F32, tag="negm_r")
            nc.vector.tensor_copy(out=negm_r[:], in_=nm_ps[:1, :])
            std_r = pA.tile([1, 128], F32, tag="std_r")
            nc.vector.tensor_copy(out=std_r[:], in_=st_ps[:1, :])

            for nm, wmat, r1m, b2m in (("q", wq_s, r1q_s, bq2_s),
                                       ("k", wk_s, r1k_s, bk2_s),
                                       ("v", wv_s, r1v_s, bv2_s)):
                ps = pAp.tile([128, 128], F32, tag="ps")
                nc.tensor.matmul(out=ps[:], lhsT=xTw[:], rhs=wmat[:],
                                 start=True, stop=False)
                nc.tensor.matmul(out=ps[:], lhsT=negm_r[:], rhs=r1m[:],
                                 start=False, stop=False)
                nc.tensor.matmul(out=ps[:], lhsT=std_r[:], rhs=b2m[:],
                                 start=False, stop=True)
                if nm == "v":
                    nc.scalar.activation(out=v_res[:, w * 128:(w + 1) * 128],
                                         in_=ps[:],
                                         func=mybir.ActivationFunctionType.Copy,
                                         scale=rstd[:])
                    nc.sync.dma_start(
                        kv_own_d[w * 128:(w + 1) * 128, 128:256],
                        v_res[:, w * 128:(w + 1) * 128])
                else:
                    ot = pA.tile([128, 128], BF16, tag=f"o_{nm}")
                    nc.scalar.activation(out=ot[:], in_=ps[:],
                                         func=mybir.ActivationFunctionType.Copy,
                                         scale=rstd[:])
                    if nm == "q":
                        nc.sync.dma_start(q_own_d[w * 128:(w + 1) * 128, :],
                                          ot[:])
                    else:
                        nc.sync.dma_start(
                            kv_own_d[w * 128:(w + 1) * 128, 0:128], ot[:])

    # AllGather packed k|v (bf16)
    if not _abl_no_coll:
        nc.gpsimd.collective_compute(
            "AllGather", mybir.AluOpType.bypass, replica_groups=rg,
            ins=[kv_own_d.opt()], outs=[kv_full.opt()])

    # ---------------- Pass 1 ----------------
    T1W = pp.T1W
    B1 = 16
    _psd_cur = [None, None]
    _kslab_cur = [None, None]
    nb1 = (T1 + B1 - 1) // B1
    with tc.tile_pool(name="p1", bufs=2) as p1, \
         tc.tile_pool(name="p1b", bufs=2) as p1b, \
         tc.tile_pool(name="pKS", bufs=4) as pKS, \
         tc.tile_pool(name="pKG", bufs=2, space="PSUM") as pKG, \
         tc.tile_pool(name="p1p", bufs=2, space="PSUM") as p1p:
        for bi in range(nb1):
            t0 = bi * B1
            nt = min(B1, T1 - t0)
            cvb = p1b.tile([128, B1 * 128], BF16, tag="cvb")
            nc.sync.dma_start(cvb[:, :nt * 128],
                              t["curv1t"][:, t0 * 128:(t0 + nt) * 128])
            qgb = p1b.tile([128, B1 * 128], BF16, tag="qgb")
            kgb = p1b.tile([128, B1 * 128], BF16, tag="kgb")
            for j in range(nt):
                ind_dma(qgb[:, j * 128:(j + 1) * 128], q_own_d[:],
                        qi_s[:, t0 + j:t0 + j + 1])
            otl = p1b.tile([128, B1 * 128], BF16, tag="otl")
            nc.sync.dma_start(otl[:, :nt * 128],
                              t["ohT_lo"][:, t0 * 128:(t0 + nt) * 128])
            oth = p1b.tile([128, B1 * 128], BF16, tag="oth")
            nc.sync.dma_start(oth[:, :nt * 128],
                              t["ohT_hi"][:, t0 * 128:(t0 + nt) * 128])
            for j in range(nt):
                ti = t0 + j
                if ti % T1W == 0:
                    ww = ti // T1W
                    sl = pKS.tile([128, 128], BF16, tag="slab_lo", name="slab_lo")
                    nc.sync.dma_start(
                        sl[:], kv_full[ww * 256:ww * 256 + 128, 0:128])
                    sh = pKS.tile([128, 128], BF16, tag="slab_hi", name="slab_hi")
                    nc.sync.dma_start(
                        sh[:], kv_full[ww * 256 + 128:(ww + 1) * 256, 0:128])
                    _kslab_cur[0] = sl
                    _kslab_cur[1] = sh
                kg = pKG.tile([128, 128], F32, tag="kg")
                nc.tensor.matmul(out=kg[:], lhsT=otl[:, j * 128:(j + 1) * 128],
                                 rhs=_kslab_cur[0][:], start=True, stop=False)
                nc.tensor.matmul(out=kg[:], lhsT=oth[:, j * 128:(j + 1) * 128],
                                 rhs=_kslab_cur[1][:], start=False, stop=True)
                nc.vector.tensor_copy(out=kgb[:, j * 128:(j + 1) * 128],
                                      in_=kg[:])
            # curv @ Wc (per-tile lhsT) into one PSUM block
            psc = p1p.tile([128, B1 * H], F32, tag="psc")
            for j in range(nt):
                nc.tensor.matmul(out=psc[:, j * H:(j + 1) * H],
                                 lhsT=cvb[:, j * 128:(j + 1) * 128],
                                 rhs=wc_s[:], start=True, stop=True)
            # scores for the whole block
            prod = p1.tile([128, B1 * 128], BF16, tag="prod")
            nc.vector.tensor_tensor(out=prod[:, :nt * 128],
                                    in0=qgb[:, :nt * 128],
                                    in1=kgb[:, :nt * 128],
                                    op=mybir.AluOpType.mult)
            qk = p1.tile([128, B1 * H], F32, tag="qk")
            nc.vector.tensor_reduce(
                out=qk[:, :nt * H],
                in_=prod[:, :nt * 128].rearrange("p (q x) -> p q x", x=HD),
                axis=mybir.AxisListType.X, op=mybir.AluOpType.add)
            qks = p1.tile([128, B1 * H], F32, tag="qks")
            nc.vector.scalar_tensor_tensor(out=qks[:, :nt * H],
                                           in0=qk[:, :nt * H],
                                           scalar=0.25, in1=psc[:, :nt * H],
                                           op0=mybir.AluOpType.mult,
                                           op1=mybir.AluOpType.add)
            nc.vector.tensor_tensor(
                out=qks[:, :nt * H].rearrange("p (q h) -> p q h", h=H),
                in0=qks[:, :nt * H].rearrange("p (q h) -> p q h", h=H),
                in1=bc_s[:].rearrange("p (o h) -> p o h", o=1)
                .broadcast_to([128, nt, H]),
                op=mybir.AluOpType.add)
            nc.scalar.activation(out=ex_sb[:, t0 * H:(t0 + nt) * H],
                                 in_=qks[:, :nt * H],
                                 func=mybir.ActivationFunctionType.Exp)
            # one-hot columns for this block
            ohb = p1.tile([128, B1 * 256], BF16, tag="ohb")
            nc.vector.tensor_tensor(
                out=ohb[:, :nt * 256].rearrange("p (q n) -> p q n", n=256),
                in0=trel_s[:, t0:t0 + nt].rearrange("p (q o) -> p q o", o=1)
                .broadcast_to([128, nt, 256]),
                in1=iota256_s[:].rearrange("p (o n) -> p o n", o=1)
                .broadcast_to([128, nt, 256]),
                op=mybir.AluOpType.is_equal)
            for j in range(nt):
                ti = t0 + j
                ex_t = ex_sb[:, ti * H:(ti + 1) * H]
                ww = ti // T1W
                tt1 = ti % T1W
                if tt1 == 0:
                    _psd_cur[0] = p1p.tile([128, H], F32, tag="psd_lo", name="psd_lo")
                    _psd_cur[1] = p1p.tile([128, H], F32, tag="psd_hi", name="psd_hi")
                psd_lo, psd_hi = _psd_cur[0], _psd_cur[1]
                nc.tensor.matmul(out=psd_lo[:],
                                 lhsT=ohb[:, j * 256:j * 256 + 128], rhs=ex_t,
                                 start=(tt1 == 0), stop=(tt1 == T1W - 1))
                nc.tensor.matmul(out=psd_hi[:],
                                 lhsT=ohb[:, j * 256 + 128:(j + 1) * 256],
                                 rhs=ex_t, start=(tt1 == 0),
                                 stop=(tt1 == T1W - 1))
                if tt1 == T1W - 1:
                    nc.vector.tensor_copy(
                        out=den_tab[:, ww * 2 * H:ww * 2 * H + H],
                        in_=psd_lo[:])
                    nc.vector.tensor_copy(
                        out=den_tab[:, ww * 2 * H + H:(ww + 1) * 2 * H],
                        in_=psd_hi[:])
        nc.sync.dma_start(
            den_d[:].rearrange("(w p) h -> p w h", p=128),
            den_tab[:, :NW * H].rearrange("p (w h) -> p w h", h=H))

    # AllReduce denom -> full table on every core
    if not _abl_no_coll:
        nc.gpsimd.collective_compute(
            "AllReduce", mybir.AluOpType.add, replica_groups=rg,
            ins=[den_d.opt()], outs=[den_full_d.opt()])

    # ---------------- Pass 1.5: messages in pass-1 order ----------------
    # msg[slot] = ex[slot] * v[tgt_slot] / den[tgt_slot]; v and 1/den
    # gathered by the same streamed one-hots as the k-gather.
    with tc.tile_pool(name="pRC", bufs=1) as pRC, \
         tc.tile_pool(name="p15", bufs=2) as p15, \
         tc.tile_pool(name="p15b", bufs=2) as p15b, \
         tc.tile_pool(name="pVS", bufs=4) as pVS, \
         tc.tile_pool(name="pVG", bufs=2, space="PSUM") as pVG:
        den_res = pRC.tile([128, NW * H], F32, name="den_res")
        nc.sync.dma_start(den_res[:].rearrange("p (w h) -> p w h", h=H),
                          den_full_d[:].rearrange("(w p) h -> p w h", p=128))
        nc.vector.tensor_scalar_max(den_res[:], den_res[:], 1e-30)
        rec_res = pRC.tile([128, NW * H], F32, name="rec_res")
        nc.vector.reciprocal(out=rec_res[:], in_=den_res[:])
        rec_b = pRC.tile([128, NW * H], BF16, name="rec_b")
        nc.vector.tensor_copy(out=rec_b[:], in_=rec_res[:])
        _vslab_cur = [None, None]
        for bi in range(0 if _abl_no_p15 else nb1):
            t0 = bi * B1
            nt = min(B1, T1 - t0)
            otl2 = p15b.tile([128, B1 * 128], BF16, tag="otl2")
            nc.sync.dma_start(otl2[:, :nt * 128],
                              t["ohT_lo"][:, t0 * 128:(t0 + nt) * 128])
            oth2 = p15b.tile([128, B1 * 128], BF16, tag="oth2")
            nc.sync.dma_start(oth2[:, :nt * 128],
                              t["ohT_hi"][:, t0 * 128:(t0 + nt) * 128])
            msgb = p15.tile([128, B1 * 128], BF16, tag="msgb15")
            prb = p15.tile([128, B1 * H], BF16, tag="prb")
            for j in range(nt):
                ti = t0 + j
                if ti % T1W == 0:
                    ww = ti // T1W
                    # fused [v-slab | 1/den-cols] rhs (136 cols)
                    vsl = pVS.tile([128, 136], BF16, tag="vslab_lo",
                                   name="vslab_lo")
                    nc.sync.dma_start(
                        vsl[:, 0:128],
                        kv_full[ww * 256:ww * 256 + 128, 128:256])
                    nc.vector.tensor_copy(
                        out=vsl[:, 128:136],
                        in_=rec_b[:, (2 * ww) * H:(2 * ww + 1) * H])
                    vsh = pVS.tile([128, 136], BF16, tag="vslab_hi",
                                   name="vslab_hi")
                    nc.sync.dma_start(
                        vsh[:, 0:128],
                        kv_full[ww * 256 + 128:(ww + 1) * 256, 128:256])
                    nc.vector.tensor_copy(
                        out=vsh[:, 128:136],
                        in_=rec_b[:, (2 * ww + 1) * H:(2 * ww + 2) * H])
                    _vslab_cur[0] = vsl
                    _vslab_cur[1] = vsh
                vgr = pVG.tile([128, 136], F32, tag="vgr")
                nc.tensor.matmul(out=vgr[:],
                                 lhsT=otl2[:, j * 128:(j + 1) * 128],
                                 rhs=_vslab_cur[0][:], start=True, stop=False)
                nc.tensor.matmul(out=vgr[:],
                                 lhsT=oth2[:, j * 128:(j + 1) * 128],
                                 rhs=_vslab_cur[1][:], start=False, stop=True)
                ti8 = ti * H
                nc.vector.tensor_tensor(out=prb[:, j * H:(j + 1) * H],
                                        in0=ex_sb[:, ti8:ti8 + H],
                                        in1=vgr[:, 128:136],
                                        op=mybir.AluOpType.mult)
                nc.vector.tensor_tensor(
                    out=msgb[:, j * 128:(j + 1) * 128]
                    .rearrange("p (h x) -> p h x", h=H),
                    in0=vgr[:, 0:128].rearrange("p (h x) -> p h x", h=H),
                    in1=prb[:, j * H:(j + 1) * H].broadcast_to([128, H, HD]),
                    op=mybir.AluOpType.mult)
            nc.sync.dma_start(
                msg_d[:].rearrange("(p t2) e -> p (t2 e)", p=128)
                [:, t0 * 128:(t0 + nt) * 128],
                msgb[:, :nt * 128])

    # ---------------- Pass 2 + Phase D ----------------
    msg_flat2 = msg_d[:]
    B2 = 16
    with tc.tile_pool(name="p2", bufs=2) as p2, \
         tc.tile_pool(name="p2b", bufs=2) as p2b, \
         tc.tile_pool(name="p2p", bufs=2, space="PSUM") as p2p, \
         tc.tile_pool(name="pD", bufs=2) as pD, \
         tc.tile_pool(name="pDp", bufs=1, space="PSUM") as pDp:
        nb2 = (T2 + B2 - 1) // B2
        # prefetch loop is flat over tiles; window boundaries align since
        # T2W*W tiles total and windows are contiguous runs of T2W tiles.
        for bi in range(nb2):
            t0 = bi * B2
            nt = min(B2, T2 - t0)
            # padded slots point at pad pass-1 rows whose msg is exactly 0,
            # so no mask multiply is needed.
            mgb = p2b.tile([128, B2 * 128], BF16, tag="mgb")
            for j in range(0, nt, 2):
                ind_dma(mgb[:, j * 128:(j + 2) * 128], msg_flat2,
                        expos_s[:, (t0 + j) // 2:(t0 + j) // 2 + 1])
            # one-hot src columns for the block
            oh2b = p2.tile([128, B2 * 128], BF16, tag="oh2b")
            nc.vector.tensor_tensor(
                out=oh2b[:, :nt * 128].rearrange("p (q n) -> p q n", n=128),
                in0=srcl2_s[:, t0:t0 + nt].rearrange("p (q o) -> p q o", o=1)
                .broadcast_to([128, nt, 128]),
                in1=iota128_s[:].rearrange("p (o n) -> p o n", o=1)
                .broadcast_to([128, nt, 128]),
                op=mybir.AluOpType.is_equal)
            for j in range(nt):
                ti = t0 + j
                w = ti // T2W
                tt = ti % T2W
                if tt == 0:
                    aggT = p2p.tile([128, 128], F32, tag="aggT")
                    tc._aggT_cur = aggT  # stash
                aggT = tc._aggT_cur
                nc.tensor.matmul(out=aggT[:],
                                 lhsT=mgb[:, j * 128:(j + 1) * 128],
                                 rhs=oh2b[:, j * 128:(j + 1) * 128],
                                 start=(tt == 0), stop=(tt == T2W - 1))
                if tt == T2W - 1:
                    # -------- Phase D for window w --------
                    aggT_sb = pD.tile([128, 128], BF16, tag="aggT_sb")
                    nc.vector.tensor_copy(out=aggT_sb[:], in_=aggT[:])
                    attn = pDp.tile([128, 128], F32, tag="attn")
                    nc.tensor.matmul(out=attn[:], lhsT=aggT_sb[:], rhs=wo_s[:],
                                     start=True, stop=False)
                    nc.tensor.matmul(out=attn[:], lhsT=ones_s[:], rhs=bo_s[:],
                                     start=False, stop=True)
                    xw2 = pD.tile([128, 128], F32, tag="xw2")
                    nc.sync.dma_start(xw2[:],
                                      t["x_own"][w * 128:(w + 1) * 128, :])
                    x1 = pD.tile([128, 128], F32, tag="x1")
                    nc.vector.tensor_tensor(out=x1[:], in0=xw2[:], in1=attn[:],
                                            op=mybir.AluOpType.add)
                    # LN2 stats
                    s1b = pD.tile([128, 1], F32, tag="s1b")
                    nc.vector.tensor_reduce(out=s1b[:], in_=x1[:],
                                            axis=mybir.AxisListType.X,
                                            op=mybir.AluOpType.add)
                    sqb = pD.tile([128, 128], F32, tag="sqb")
                    nc.scalar.activation(
                        out=sqb[:], in_=x1[:],
                        func=mybir.ActivationFunctionType.Square)
                    s2b = pD.tile([128, 1], F32, tag="s2b")
                    nc.vector.tensor_reduce(out=s2b[:], in_=sqb[:],
                                            axis=mybir.AxisListType.X,
                                            op=mybir.AluOpType.add)
                    mb = pD.tile([128, 1], F32, tag="mb")
                    nc.vector.tensor_scalar_mul(mb[:], s1b[:], 1.0 / 128.0)
                    m2b = pD.tile([128, 1], F32, tag="m2b")
                    nc.vector.tensor_tensor(out=m2b[:], in0=mb[:], in1=mb[:],
                                            op=mybir.AluOpType.mult)
                    varb = pD.tile([128, 1], F32, tag="varb")
                    nc.vector.scalar_tensor_tensor(
                        out=varb[:], in0=s2b[:], scalar=1.0 / 128.0, in1=m2b[:],
                        op0=mybir.AluOpType.mult, op1=mybir.AluOpType.subtract)
                    stdb = pD.tile([128, 1], F32, tag="stdb")
                    nc.scalar.activation(
                        out=stdb[:], in_=varb[:],
                        func=mybir.ActivationFunctionType.Sqrt,
                        bias=eps_col[:])
                    rstdb = pD.tile([128, 1], F32, tag="rstdb")
                    nc.vector.reciprocal(out=rstdb[:], in_=stdb[:])
                    negmb = pD.tile([128, 1], F32, tag="negmb")
                    nc.vector.tensor_scalar_mul(negmb[:], mb[:], -1.0)
                    nm_psb = pDp.tile([128, 128], F32, tag="tr_psb")
                    nc.tensor.transpose(out=nm_psb[:1, :], in_=negmb[:],
                                        identity=ident[:])
                    st_psb = pDp.tile([128, 128], F32, tag="tr_psb")
                    nc.tensor.transpose(out=st_psb[:1, :], in_=stdb[:],
                                        identity=ident[:])
                    negm_rb = pD.tile([1, 128], BF16, tag="negm_rb")
                    nc.vector.tensor_copy(out=negm_rb[:], in_=nm_psb[:1, :])
                    std_rb = pD.tile([1, 128], BF16, tag="std_rb")
                    nc.vector.tensor_copy(out=std_rb[:], in_=st_psb[:1, :])
                    # x1T (bf16 for the FFN matmuls)
                    x1T_ps = pDp.tile([128, 128], F32, tag="tr_psb")
                    nc.tensor.transpose(out=x1T_ps[:], in_=x1[:],
                                        identity=ident[:])
                    x1T = pD.tile([128, 128], BF16, tag="x1T")
                    nc.vector.tensor_copy(out=x1T[:], in_=x1T_ps[:])
                    hp = pDp.tile([128, 512], F32, tag="hp")
                    nc.tensor.matmul(out=hp[:], lhsT=x1T[:], rhs=w1_s[:],
                                     start=True, stop=False)
                    nc.tensor.matmul(out=hp[:], lhsT=negm_rb[:], rhs=r11_s[:],
                                     start=False, stop=False)
                    nc.tensor.matmul(out=hp[:], lhsT=std_rb[:], rhs=b12_s[:],
                                     start=False, stop=True)
                    hsb = pD.tile([128, 512], BF16, tag="hsb")
                    nc.scalar.activation(out=hsb[:], in_=hp[:],
                                         func=mybir.ActivationFunctionType.Relu,
                                         scale=rstdb[:])
                    ffn = pDp.tile([128, 128], F32, tag="ffn")
                    for cch in range(4):
                        hT_ps = pDp.tile([128, 128], BF16, tag="tr_psb2")
                        nc.tensor.transpose(
                            out=hT_ps[:], in_=hsb[:, cch * 128:(cch + 1) * 128],
                            identity=ident_b[:])
                        hT = pD.tile([128, 128], BF16, tag="hT")
                        nc.vector.tensor_copy(out=hT[:], in_=hT_ps[:])
                        nc.tensor.matmul(out=ffn[:], lhsT=hT[:],
                                         rhs=w2_s[:, cch * 128:(cch + 1) * 128],
                                         start=(cch == 0), stop=False)
                    nc.tensor.matmul(out=ffn[:], lhsT=ones_s[:], rhs=b2_s[:],
                                     start=False, stop=True)
                    outw = pD.tile([128, 128], F32, tag="outw")
                    nc.vector.tensor_tensor(out=outw[:], in0=x1[:], in1=ffn[:],
                                            op=mybir.AluOpType.add)
                    nc.sync.dma_start(t["out"][w * 128:(w + 1) * 128, :],
                                      outw[:])

    ctx.close()


def build_program(pp, nc_factory):
    """Create Bacc, declare IO, build tile program, compile. Returns nc."""
    import concourse.tile as tile
    nc = nc_factory()
    t = declare_io(nc, pp)
    with tile.TileContext(nc) as tc:
        build(tc, t, pp)
    nc.compile()
    return nc


# --------------------------------------------------------------------------
# Harness entry point
# --------------------------------------------------------------------------

NCORES = 8
W_PER_CORE = 49  # 8*49*128 = 50176 >= 50000 nodes


def _run_spmd_timed(nc, in_maps, n_cores, reps=14, chain=8):
    """Execute the SPMD program via PJRT with device-staged inputs; returns
    (per-core results, steady-state per-execution time in ns).

    The axon tunnel adds ~75 ms of fixed dispatch round-trip latency per
    synchronous call, unrelated to on-device execution. We measure T(1) and
    T(1+chain) where the extra executions are chained back-to-back on device
    (each feeding its output buffer to the next call), and report
    (T(1+chain) - T(1)) / chain: the marginal hardware execution time.
    """
    import time

    import jax
    from jax.experimental.shard_map import shard_map
    from jax.sharding import Mesh, NamedSharding, PartitionSpec

    from concourse.bass2jax import (_bass_exec_p, install_neuronx_cc_hook,
                                    partition_id_tensor)

    install_neuronx_cc_hook()
    partition_name = (nc.partition_id_tensor.name
                      if nc.partition_id_tensor else None)
    in_names, out_names, out_avals, zero_outs = [], [], [], []
    for alloc in nc.m.functions[0].allocations:
        if not isinstance(alloc, mybir.MemoryLocationSet):
            continue
        name = alloc.memorylocations[0].name
        if alloc.kind == "ExternalInput":
            if name != partition_name:
                in_names.append(name)
        elif alloc.kind == "ExternalOutput":
            shape = tuple(alloc.tensor_shape)
            dtype = mybir.dt.np(alloc.dtype)
            out_names.append(name)
            out_avals.append(jax.core.ShapedArray(shape, dtype))
            zero_outs.append(np.zeros(shape, dtype))
    n_params = len(in_names)
    n_outs = len(out_avals)
    in_names.extend(out_names)
    if partition_name is not None:
        in_names.append(partition_name)
    donate = tuple(range(n_params, n_params + n_outs))

    def _body(*args):
        operands = list(args)
        if partition_name is not None:
            operands.append(partition_id_tensor())
        outs = _bass_exec_p.bind(
            *operands, out_avals=tuple(out_avals), in_names=tuple(in_names),
            out_names=tuple(out_names), lowering_input_output_aliases=(),
            sim_require_finite=True, sim_require_nnan=True, nc=nc)
        return tuple(outs)

    devices = jax.devices()[:n_cores]
    mesh = Mesh(np.asarray(devices), ("core",))
    sharding = NamedSharding(mesh, PartitionSpec("core"))
    in_specs = (PartitionSpec("core"),) * (n_params + n_outs)
    out_specs = (PartitionSpec("core"),) * len(out_names)
    sharded = jax.jit(
        shard_map(_body, mesh=mesh, in_specs=in_specs, out_specs=out_specs,
                  check_rep=False),
        donate_argnums=donate, keep_unused=True)
    concat_in = [
        np.concatenate([np.asarray(in_maps[c][in_names[i]])
                        for c in range(n_cores)], axis=0)
        for i in range(n_params)]
    dev_in = [jax.device_put(a, sharding) for a in concat_in]

    def fresh_zeros():
        zs = [jax.device_put(
            np.zeros((n_cores * z.shape[0], *z.shape[1:]), z.dtype), sharding)
            for z in zero_outs]
        jax.block_until_ready(zs)
        return zs

    out_arrs = sharded(*dev_in, *fresh_zeros())
    jax.block_until_ready(out_arrs)
    results = [
        {name: np.asarray(out_arrs[i]).reshape(n_cores, *out_avals[i].shape)[c]
         for i, name in enumerate(out_names)}
        for c in range(n_cores)]

    def run_chain(n_execs):
        o = tuple(fresh_zeros())
        t0 = time.perf_counter()
        for _ in range(n_execs):
            o = sharded(*dev_in, *o)
        jax.block_until_ready(o)
        return time.perf_counter() - t0

    best = None
    for _ in range(max(reps, 0)):
        t_one = run_chain(1)
        t_many = run_chain(1 + chain)
        marginal = (t_many - t_one) / chain
        best = marginal if best is None or marginal < best else best
    return results, (None if best is None else int(best * 1e9))


def kernel(**inputs):
    import sys
    if "/opt/trn_rl_repo" not in sys.path:
        sys.path.insert(0, "/opt/trn_rl_repo")
    import concourse.bacc as bacc

    x = np.asarray(inputs["x"], np.float32)
    edge_index = np.asarray(inputs["edge_index"])
    curv = np.asarray(inputs["curvature_embeddings"], np.float32)
    weights = {k: np.asarray(v) for k, v in inputs.items()
               if k not in ("x", "edge_index", "curvature_embeddings")}

    pp, in_maps = host_prep(x, edge_index, curv, weights, NCORES, W_PER_CORE)
    nc = build_program(pp, lambda: bacc.Bacc(
        "TRN2", target_bir_lowering=False, debug=False, num_devices=NCORES,
        num_swdge_queues=4))
    results, best_ns = _run_spmd_timed(nc, in_maps, NCORES)
    kernel.last_exec_ns = best_ns
    out = np.concatenate([results[c]["out"] for c in range(NCORES)],
                         axis=0)[:x.shape[0]]
    return np.ascontiguousarray(out, dtype=np.float32)
